# revision 11
# baseline (speedup 1.0000x reference)
"""Trainium2 Bass kernel for nn_DynamicGraphNet (2-layer GNN attention message passing).

Contract: kernel(**inputs) takes the FULL unsharded inputs (as produced by the
reference's setup_inputs) and returns the full output (output_vec[64], x[2176, 64]).

Strategy: the reference's edge_index is a deterministic complete-bipartite
block graph (64 input -> 2048 hidden -> 64 output).  We verify that structure
at runtime; when it holds, gather/scatter collapses into dense per-head
matmuls.  The 2048 hidden nodes are sharded across 8 NeuronCores (256 each);
the global edge-softmax and the output-node scatter-add are handled with one
small AllGather per layer (softmax max/sum stats + per-core partial
aggregates for the 64 output nodes).  If the structure check fails, a general
jax fallback implements the exact reference math for arbitrary graphs.
"""

import functools
import os
import sys

import numpy as np

for _p in ("/root/.axon_site", "/root/.axon_site/_ro/trn_rl_repo",
           "/root/.axon_site/_ro/pypackages", "/opt/trn_rl_repo"):
    if os.path.isdir(_p) and _p not in sys.path:
        sys.path.append(_p)

N_NODES = 2176
N_IN = 64
N_HID = 2048
N_OUT = 64
D = 64
H = 4
HD = D * H  # 256
E1 = N_IN * N_HID
E2 = N_HID * N_OUT
E = E1 + E2
NCORES = 8
J = N_HID // NCORES  # 256 hidden nodes per core
NB = 384  # nodes held per core: J chunk + N_IN + N_OUT
PAY = 8 + 2 * 128 * 64  # collective payload floats per core: stats + 2 agg tiles


def _expected_edge_index() -> np.ndarray:
    hs, os_ = N_IN, N_IN + N_HID
    s1 = np.repeat(np.arange(0, N_IN), N_HID)
    t1 = np.tile(np.arange(hs, os_), N_IN)
    s2 = np.repeat(np.arange(hs, os_), N_OUT)
    t2 = np.tile(np.arange(os_, N_NODES), N_HID)
    return np.stack([np.concatenate([s1, s2]), np.concatenate([t1, t2])]).astype(np.int32)


# ---------------------------------------------------------------------------
# Bass program
# ---------------------------------------------------------------------------

def _build_program():
    import concourse.bacc as bacc
    import concourse.mybir as mybir
    from concourse import masks
    from concourse.tile import TileContext

    F32 = mybir.dt.float32
    AF = mybir.ActivationFunctionType
    ALU = mybir.AluOpType
    AX = mybir.AxisListType

    nc = bacc.Bacc("TRN2", target_bir_lowering=False, debug=False,
                   num_devices=NCORES)

    # ---- I/O ----
    xT_d = nc.dram_tensor("xT", [D, NB], F32, kind="ExternalInput")
    ew1_d = nc.dram_tensor("ew1", [N_IN, J], F32, kind="ExternalInput")
    ew2p_d = nc.dram_tensor("ew2p", [128, 128], F32, kind="ExternalInput")
    wq_d, wks_d, wv_d, wo_d, we_d, bo_d = [], [], [], [], [], []
    for l in (1, 2):
        wq_d.append(nc.dram_tensor(f"wq{l}", [D, HD], F32, kind="ExternalInput"))
        wks_d.append(nc.dram_tensor(f"wks{l}", [D, HD], F32, kind="ExternalInput"))
        wv_d.append(nc.dram_tensor(f"wv{l}", [D, HD], F32, kind="ExternalInput"))
        wo_d.append(nc.dram_tensor(f"wo{l}", [HD, D], F32, kind="ExternalInput"))
        we_d.append(nc.dram_tensor(f"we{l}", [1, H], F32, kind="ExternalInput"))
        bo_d.append(nc.dram_tensor(f"bo{l}", [D, 1], F32, kind="ExternalInput"))
    wproj_d = nc.dram_tensor("wproj", [D, 1], F32, kind="ExternalInput")
    bproj_d = nc.dram_tensor("bproj", [1, 1], F32, kind="ExternalInput")

    xoutT_d = nc.dram_tensor("xoutT", [D, NB], F32, kind="ExternalOutput")
    outvec_d = nc.dram_tensor("outvec", [N_OUT, 1], F32, kind="ExternalOutput")

    with TileContext(nc) as tc:
        with (
            tc.tile_pool(name="consts", bufs=1) as cp,
            tc.tile_pool(name="wts", bufs=1) as wpool,
            tc.tile_pool(name="work", bufs=2) as wk,
            tc.tile_pool(name="ps", bufs=1, space="PSUM") as ps,
            tc.tile_pool(name="dram", bufs=1, space="DRAM") as dr,
        ):
            # ---- constants ----
            ident = cp.tile([128, 128], F32)
            masks.make_identity(nc, ident[:])
            ones = cp.tile([1, 128], F32)
            nc.vector.memset(ones[:], 1.0)
            # head-expansion one-hots: expA[k, m]=1 iff m//64==k (k<2); expB for k-2
            expA_cols = cp.tile([128, H], F32)
            expB_cols = cp.tile([128, H], F32)
            nc.vector.memset(expA_cols[:], 0.0)
            nc.vector.memset(expB_cols[:], 0.0)
            nc.vector.memset(expA_cols[0:64, 0:1], 1.0)
            nc.vector.memset(expA_cols[64:128, 1:2], 1.0)
            nc.vector.memset(expB_cols[0:64, 2:3], 1.0)
            nc.vector.memset(expB_cols[64:128, 3:4], 1.0)

            expA_p = ps.tile([H, 128], F32, tag="small", bufs=3, name="expA_p")
            expB_p = ps.tile([H, 128], F32, tag="small", bufs=3, name="expB_p")
            nc.tensor.transpose(expA_p[:], expA_cols[:], ident[:])
            nc.tensor.transpose(expB_p[:], expB_cols[:], ident[:])
            expA = cp.tile([H, 128], F32)
            expB = cp.tile([H, 128], F32)
            nc.vector.tensor_copy(expA[:], expA_p[:])
            nc.vector.tensor_copy(expB[:], expB_p[:])

            # ---- load inputs ----
            def load(pool, dram, shape):
                t = pool.tile(shape, F32, tag=dram.name, name=dram.name)
                nc.sync.dma_start(out=t[:], in_=dram[:])
                return t

            XT = load(wpool, xT_d, [D, NB])
            EW1 = load(wpool, ew1_d, [N_IN, J])
            EW2P = load(wpool, ew2p_d, [128, 128])
            WQ = [load(wpool, wq_d[i], [D, HD]) for i in range(2)]
            WKS = [load(wpool, wks_d[i], [D, HD]) for i in range(2)]
            WV = [load(wpool, wv_d[i], [D, HD]) for i in range(2)]
            WE = [load(wpool, we_d[i], [1, H]) for i in range(2)]
            BO = [load(wpool, bo_d[i], [D, 1]) for i in range(2)]
            WO0, WO1 = [], []
            for i in range(2):
                t0 = wpool.tile([128, D], F32, tag=f"wo{i}a", name=f"wo{i}a")
                t1 = wpool.tile([128, D], F32, tag=f"wo{i}b", name=f"wo{i}b")
                nc.sync.dma_start(out=t0[:], in_=wo_d[i][0:128, :])
                nc.sync.dma_start(out=t1[:], in_=wo_d[i][128:256, :])
                WO0.append(t0)
                WO1.append(t1)
            WPRJ = load(wpool, wproj_d, [D, 1])
            BPRJ = load(wpool, bproj_d, [1, 1])

            # collective bounce buffers (per layer)
            ccin = [dr.tile([1, PAY], F32, tag=f"ccin{l}", name=f"ccin{l}") for l in range(2)]
            ccout = [dr.tile([NCORES, PAY], F32, tag=f"ccout{l}", name=f"ccout{l}") for l in range(2)]

            stage = os.environ.get("GNN_STAGE", "full")

            def layer(li, XT):
                """One message-passing layer; returns new XT [D, NB] tile."""
                wq, wks, wv, we, bo = WQ[li], WKS[li], WV[li], WE[li], BO[li]
                wo0, wo1 = WO0[li], WO1[li]

                # ---- projections ----
                # Per-head QT/KT [64, NB] so every matmul operand sits at
                # base partition 0 (HW rejects mixed operand bases within a
                # PSUM tile's accumulation groups).
                QThp = [ps.tile([D, NB], F32, tag="qkv", bufs=2, name=f"qt_p{h}")
                        for h in range(H)]
                KThp = [ps.tile([D, NB], F32, tag="qkv", bufs=2, name=f"kt_p{h}")
                        for h in range(H)]
                for h in range(H):
                    nc.tensor.matmul(QThp[h][:], wq[:, 64 * h:64 * h + 64], XT[:])
                    nc.tensor.matmul(KThp[h][:], wks[:, 64 * h:64 * h + 64], XT[:])
                # V node-major [NB, HD] (3 partition tiles: chunk0, chunk1, I+O)
                Vp = [ps.tile([128, NB], F32, tag="qkv", bufs=2, name=f"v_p{t}")[:, 0:HD] for t in range(3)]
                for t in range(3):
                    nc.tensor.matmul(Vp[t][:], XT[:, 128 * t:128 * (t + 1)], wv[:])
                QTh = [wk.tile([D, NB], F32, tag=f"qth{h}", name=f"qth{h}") for h in range(H)]
                KTh = [wk.tile([D, NB], F32, tag=f"kth{h}", name=f"kth{h}") for h in range(H)]
                V = [wk.tile([128, HD], F32, tag=f"v{t}", name=f"v{t}") for t in range(3)]
                for h in range(H):
                    nc.vector.tensor_copy(QTh[h][:], QThp[h][:])
                    nc.scalar.copy(KTh[h][:], KThp[h][:])
                for t in range(3):
                    nc.vector.tensor_copy(V[t][:], Vp[t][:])

                if li == 0 and stage == "proj":
                    nc.sync.dma_start(out=xoutT_d[:], in_=QTh[0][:, :])
                    nc.sync.dma_start(out=outvec_d[:], in_=V[0][0:64, 0:1])
                    return None

                def qt_h(h, cols):
                    return QTh[h][:, cols]

                def kt_h(h, cols):
                    return KTh[h][:, cols]

                CH0, CH1 = slice(0, 128), slice(128, 256)
                ICOL, OCOL = slice(256, 320), slice(320, 384)

                # ---- logits ----
                # S1[i, h*J + j] = (K_h[I] . Q_h[chunk_j]) / 8
                S1p = ps.tile([N_IN, H * J], F32, tag="s1", name="s1_p")
                for h in range(H):
                    nc.tensor.matmul(S1p[:, J * h:J * (h + 1)],
                                     kt_h(h, ICOL), qt_h(h, slice(0, J)))
                # S2[p, jh*256 + h*64 + o] = (K_h[chunk_{jh*128+p}] . Q_h[O_o]) / 8
                S2p = ps.tile([128, 512], F32, tag="s2", name="s2_p")
                for h in range(H):
                    for jh in range(2):
                        nc.tensor.matmul(
                            S2p[:, 256 * jh + 64 * h:256 * jh + 64 * h + 64],
                            kt_h(h, (CH0, CH1)[jh]), qt_h(h, OCOL))

                if li == 0 and stage == "s1s2":
                    o1 = wk.tile([64, 512], F32, tag="dbg1", name="dbg1")
                    nc.vector.tensor_copy(o1[:], S1p[:, 0:512])
                    o2 = wk.tile([128, 384], F32, tag="dbg2", name="dbg2")
                    nc.vector.tensor_copy(o2[:], S2p[:, 0:384])
                    nc.sync.dma_start(out=xoutT_d[:], in_=o1[0:64, 0:384])
                    nc.sync.dma_start(out=outvec_d[:], in_=o2[0:64, 0:1])
                    return None

                # ---- edge-weight term + add + leaky relu ----
                web_p = ps.tile([128, H], F32, tag="small", bufs=3, name="web_p")
                nc.tensor.matmul(web_p[:], ones[:], we[:])
                web = wk.tile([128, H], F32, tag="web")
                nc.vector.tensor_copy(web[:], web_p[:])

                LA = wk.tile([N_IN, H * J], F32, tag="la")
                LB = wk.tile([128, 512], F32, tag="lb")
                for h in range(H):
                    nc.vector.tensor_scalar_mul(
                        LA[:, J * h:J * (h + 1)], EW1[:], web[0:64, h:h + 1])
                for h in range(H):
                    for jh in range(2):
                        nc.vector.tensor_scalar_mul(
                            LB[:, 256 * jh + 64 * h:256 * jh + 64 * h + 64],
                            EW2P[:, 64 * jh:64 * jh + 64], web[:, h:h + 1])
                nc.vector.tensor_add(LA[:], LA[:], S1p[:])
                nc.vector.tensor_add(LB[:], LB[:], S2p[:])
                if li == 0 and stage == "eww":
                    nc.sync.dma_start(out=xoutT_d[:], in_=LA[0:64, 0:NB])
                    nc.sync.dma_start(out=outvec_d[:], in_=LB[0:64, 0:1])
                    return None
                # leaky_relu(x) = max(x, 0.2*x)
                LAt = wk.tile([N_IN, H * J], F32, tag="lat")
                LBt = wk.tile([128, 512], F32, tag="lbt")
                nc.scalar.activation(LAt[:], LA[:], AF.Identity, scale=0.2)
                nc.scalar.activation(LBt[:], LB[:], AF.Identity, scale=0.2)
                nc.vector.tensor_max(LA[:], LA[:], LAt[:])
                nc.vector.tensor_max(LB[:], LB[:], LBt[:])

                if li == 0 and stage == "lrelu":
                    nc.sync.dma_start(out=xoutT_d[:], in_=LA[0:64, 0:NB])
                    nc.sync.dma_start(out=outvec_d[:], in_=LB[0:64, 0:1])
                    return None
                # ---- per-partition (negated) maxes ----
                n1 = wk.tile([N_IN, H], F32, tag="n1")
                for h in range(H):
                    nc.vector.tensor_reduce(
                        out=n1[:, h:h + 1], in_=LA[:, J * h:J * (h + 1)],
                        op=ALU.max, axis=AX.X, negate=True)
                n2 = wk.tile([128, 2 * H], F32, tag="n2")
                for h in range(H):
                    for jh in range(2):
                        nc.vector.tensor_reduce(
                            out=n2[:, 4 * jh + h:4 * jh + h + 1],
                            in_=LB[:, 256 * jh + 64 * h:256 * jh + 64 * h + 64],
                            op=ALU.max, axis=AX.X, negate=True)

                if li == 0 and stage == "rowmax":
                    nc.sync.dma_start(out=xoutT_d[0:64, 0:4], in_=n1[:])
                    nc.sync.dma_start(out=outvec_d[0:64, 0:1], in_=n2[0:64, 0:1])
                    return None
                # ---- U = exp(L - rowmax), rowsums ----
                ULA = wk.tile([N_IN, H * J], F32, tag="ula")
                ULB = wk.tile([128, 512], F32, tag="ulb")
                rs1 = wk.tile([N_IN, H], F32, tag="rs1")
                rs2 = wk.tile([128, 2 * H], F32, tag="rs2")
                for h in range(H):
                    nc.scalar.activation(
                        ULA[:, J * h:J * (h + 1)], LA[:, J * h:J * (h + 1)],
                        AF.Exp, bias=n1[:, h:h + 1], accum_out=rs1[:, h:h + 1])
                for h in range(H):
                    for jh in range(2):
                        s = slice(256 * jh + 64 * h, 256 * jh + 64 * h + 64)
                        nc.scalar.activation(
                            ULB[:, s], LB[:, s], AF.Exp,
                            bias=n2[:, 4 * jh + h:4 * jh + h + 1],
                            accum_out=rs2[:, 4 * jh + h:4 * jh + h + 1])

                if li == 0 and stage == "exp":
                    nc.sync.dma_start(out=xoutT_d[:], in_=ULA[0:64, 0:NB])
                    nc.sync.dma_start(out=outvec_d[:], in_=rs2[0:64, 0:1])
                    return None

                # ---- core-local softmax stats (negated-max domain) ----
                # nm_c[h] = min(n1, n2) over partitions; s_c from rowsums
                n2m = wk.tile([128, H], F32, tag="n2m")
                nc.vector.tensor_tensor(n2m[:], n2[:, 0:4], n2[:, 4:8], op=ALU.min)
                n1T_p = ps.tile([H, N_IN], F32, tag="small", bufs=3, name="n1T_p")
                nc.tensor.transpose(n1T_p[:], n1[:], ident[0:64, 0:64])
                n2mT_p = ps.tile([H, 128], F32, tag="small", bufs=3, name="n2mT_p")
                nc.tensor.transpose(n2mT_p[:], n2m[:], ident[:])
                r1 = wk.tile([H, 1], F32, tag="r1")
                r2 = wk.tile([H, 1], F32, tag="r2")
                nc.vector.tensor_reduce(out=r1[:], in_=n1T_p[:], op=ALU.min, axis=AX.X)
                nc.vector.tensor_reduce(out=r2[:], in_=n2mT_p[:], op=ALU.min, axis=AX.X)
                nmc = wk.tile([H, 1], F32, tag="nmc")
                nc.vector.tensor_tensor(nmc[:], r1[:], r2[:], op=ALU.min)

                # s_c = sum(rowsum * exp(rowmax - m_c))
                rs1T_p = ps.tile([H, N_IN], F32, tag="small", bufs=3, name="rs1T_p")
                nc.tensor.transpose(rs1T_p[:], rs1[:], ident[0:64, 0:64])
                n1T_e = wk.tile([H, N_IN], F32, tag="n1te")
                nc.scalar.activation(n1T_e[:], n1T_p[:], AF.Exp,
                                     bias=nmc[:], scale=-1.0)
                pr1 = wk.tile([H, N_IN], F32, tag="pr1")
                nc.vector.tensor_mul(pr1[:], n1T_e[:], rs1T_p[:])
                sc = wk.tile([H, 1], F32, tag="sc")
                nc.vector.reduce_sum(out=sc[:], in_=pr1[:], axis=AX.X)
                for jh in range(2):
                    n2T_p = ps.tile([H, 128], F32, tag="small", bufs=3, name="n2T_p")
                    rs2T_p = ps.tile([H, 128], F32, tag="small", bufs=3, name="rs2T_p")
                    nc.tensor.transpose(n2T_p[:], n2[:, 4 * jh:4 * jh + 4], ident[:])
                    nc.tensor.transpose(rs2T_p[:], rs2[:, 4 * jh:4 * jh + 4], ident[:])
                    n2T_e = wk.tile([H, 128], F32, tag="n2te")
                    nc.scalar.activation(n2T_e[:], n2T_p[:], AF.Exp,
                                         bias=nmc[:], scale=-1.0)
                    pr2 = wk.tile([H, 128], F32, tag="pr2")
                    nc.vector.tensor_mul(pr2[:], n2T_e[:], rs2T_p[:])
                    sc2 = wk.tile([H, 1], F32, tag="sc2")
                    nc.vector.reduce_sum(out=sc2[:], in_=pr2[:], axis=AX.X)
                    nc.vector.tensor_add(sc[:], sc[:], sc2[:])

                # ---- V-hat for block 2 (scale V chunk rows by exp(rowmax - m_c)) ----
                nmcT_p = ps.tile([1, H], F32, tag="small", bufs=3, name="nmcT_p")
                nc.tensor.transpose(nmcT_p[:], nmc[:], ident[0:4, 0:4])
                row8 = wk.tile([1, 2 * H], F32, tag="row8")
                nc.vector.tensor_copy(row8[:, 0:4], nmcT_p[:])
                nc.vector.tensor_copy(row8[:, 4:8], nmcT_p[:])
                nmc8b_p = ps.tile([128, 2 * H], F32, tag="small", bufs=3, name="nmc8b_p")
                nc.tensor.matmul(nmc8b_p[:], ones[:], row8[:])
                dal2 = wk.tile([128, 2 * H], F32, tag="dal2")
                nc.vector.tensor_sub(dal2[:], n2[:], nmc8b_p[:])
                al2 = wk.tile([128, 2 * H], F32, tag="al2")
                nc.scalar.activation(al2[:], dal2[:], AF.Exp, scale=-1.0)
                Vh2 = [wk.tile([128, HD], F32, tag=f"vh2_{t}", name=f"vh2_{t}") for t in range(2)]
                for jh in range(2):
                    for h in range(H):
                        nc.vector.tensor_scalar_mul(
                            Vh2[jh][:, 64 * h:64 * h + 64],
                            V[jh][:, 64 * h:64 * h + 64],
                            al2[:, 4 * jh + h:4 * jh + h + 1])

                # ---- block-2 partial aggregation: AGG2T_h[d, o] ----
                A2p = [ps.tile([128, N_OUT], F32, tag="small", bufs=3, name=f"a2_p{i}") for i in range(2)]
                for h in range(H):
                    for jh in range(2):
                        nc.tensor.matmul(
                            A2p[h // 2][64 * (h % 2):64 * (h % 2) + 64, :],
                            Vh2[jh][:, 64 * h:64 * h + 64],
                            ULB[:, 256 * jh + 64 * h:256 * jh + 64 * h + 64],
                            start=(jh == 0), stop=(jh == 1))
                A2s = [wk.tile([128, N_OUT], F32, tag=f"a2s{i}", name=f"a2s{i}") for i in range(2)]
                nc.vector.tensor_copy(A2s[0][:], A2p[0][:])
                nc.vector.tensor_copy(A2s[1][:], A2p[1][:])

                # ---- payload DMA + AllGather ----
                ci, co = ccin[li], ccout[li]
                nc.sync.dma_start(out=ci[0:1, 0:4].rearrange("a b -> b a"), in_=nmc[:])
                nc.sync.dma_start(out=ci[0:1, 4:8].rearrange("a b -> b a"), in_=sc[:])
                nc.sync.dma_start(
                    out=ci[0:1, 8:8200].rearrange("a (p f) -> (a p) f", p=128),
                    in_=A2s[0][:])
                nc.sync.dma_start(
                    out=ci[0:1, 8200:16392].rearrange("a (p f) -> (a p) f", p=128),
                    in_=A2s[1][:])
                if li == 0 and stage == "payload":
                    nc.sync.dma_start(out=xoutT_d[:], in_=ULB[0:64, 0:NB])
                    nc.sync.dma_start(out=outvec_d[0:4, 0:1], in_=sc[:])
                    return None
                nc.gpsimd.collective_compute(
                    "AllGather", mybir.AluOpType.bypass,
                    replica_groups=[list(range(NCORES))],
                    ins=[ci.opt()], outs=[co.opt()])

                # ---- global stats ----
                Gm = wk.tile([NCORES, H], F32, tag="gm")
                Gs = wk.tile([NCORES, H], F32, tag="gs")
                nc.sync.dma_start(out=Gm[:], in_=co[:, 0:4])
                nc.sync.dma_start(out=Gs[:], in_=co[:, 4:8])
                GmT_p = ps.tile([H, NCORES], F32, tag="small", bufs=3, name="GmT_p")
                GsT_p = ps.tile([H, NCORES], F32, tag="small", bufs=3, name="GsT_p")
                nc.tensor.transpose(GmT_p[:], Gm[:], ident[0:8, 0:8])
                nc.tensor.transpose(GsT_p[:], Gs[:], ident[0:8, 0:8])
                nmg = wk.tile([H, 1], F32, tag="nmg")
                nc.vector.tensor_reduce(out=nmg[:], in_=GmT_p[:], op=ALU.min, axis=AX.X)
                # w[h, c] = exp(m_c - m_g)
                wc = wk.tile([H, NCORES], F32, tag="wc")
                nc.scalar.activation(wc[:], GmT_p[:], AF.Exp, bias=nmg[:], scale=-1.0)
                spr = wk.tile([H, NCORES], F32, tag="spr")
                nc.vector.tensor_mul(spr[:], wc[:], GsT_p[:])
                Sg = wk.tile([H, 1], F32, tag="sg")
                nc.vector.reduce_sum(out=Sg[:], in_=spr[:], axis=AX.X)
                sinv = wk.tile([H, 1], F32, tag="sinv")
                nc.vector.reciprocal(sinv[:], Sg[:])

                if li == 0 and stage == "gather":
                    nc.sync.dma_start(out=xoutT_d[0:4, 0:8], in_=wc[:])
                    nc.sync.dma_start(out=outvec_d[0:4, 0:1], in_=Sg[:])
                    return None

                # ---- gather-weighted sum of block-2 partials ----
                wexp_p = [ps.tile([128, NCORES], F32, tag="small", bufs=3, name="wexp_p0"),
                          ps.tile([128, NCORES], F32, tag="small", bufs=3, name="wexp_p1")]
                nc.tensor.matmul(wexp_p[0][:], expA[:], wc[:])
                nc.tensor.matmul(wexp_p[1][:], expB[:], wc[:])
                wexp = [wk.tile([128, NCORES], F32, tag=f"wexp{i}", name=f"wexp{i}") for i in range(2)]
                nc.vector.tensor_copy(wexp[0][:], wexp_p[0][:])
                nc.vector.tensor_copy(wexp[1][:], wexp_p[1][:])
                A2G = [wk.tile([128, N_OUT], F32, tag=f"a2g{i}", name=f"a2g{i}") for i in range(2)]
                for i in range(2):
                    off = 8 + 8192 * i
                    for c in range(NCORES):
                        part = wk.tile([128, N_OUT], F32, tag=f"part{i}", name=f"part{i}")
                        nc.sync.dma_start(
                            out=part[:],
                            in_=co[c:c + 1, off:off + 8192].rearrange(
                                "a (p f) -> (a p) f", p=128))
                        if c == 0:
                            nc.vector.tensor_scalar_mul(
                                A2G[i][:], part[:], wexp[i][:, 0:1])
                        else:
                            tmp = wk.tile([128, N_OUT], F32, tag=f"tmp{i}", name=f"tmp{i}")
                            nc.vector.tensor_scalar_mul(
                                tmp[:], part[:], wexp[i][:, c:c + 1])
                            nc.vector.tensor_add(A2G[i][:], A2G[i][:], tmp[:])

                # ---- wo scaled by 1/S_g per head-block of rows ----
                rsc_p = [ps.tile([128, 1], F32, tag="small", bufs=3, name="rsc_p0"),
                         ps.tile([128, 1], F32, tag="small", bufs=3, name="rsc_p1")]
                nc.tensor.matmul(rsc_p[0][:], expA[:], sinv[:])
                nc.tensor.matmul(rsc_p[1][:], expB[:], sinv[:])
                rsc = [wk.tile([128, 1], F32, tag=f"rsc{i}", name=f"rsc{i}") for i in range(2)]
                nc.vector.tensor_copy(rsc[0][:], rsc_p[0][:])
                nc.vector.tensor_copy(rsc[1][:], rsc_p[1][:])
                wos0 = wk.tile([128, D], F32, tag="wos0")
                wos1 = wk.tile([128, D], F32, tag="wos1")
                nc.vector.tensor_scalar_mul(wos0[:], wo0[:], rsc[0][:])
                nc.vector.tensor_scalar_mul(wos1[:], wo1[:], rsc[1][:])

                # ---- block-1 aggregation (post-gather; fold exp(rowmax-m_g) into V_I) ----
                nmgT_p = ps.tile([1, H], F32, tag="small", bufs=3, name="nmgT_p")
                nc.tensor.transpose(nmgT_p[:], nmg[:], ident[0:4, 0:4])
                nmgr = wk.tile([1, H], F32, tag="nmgr")
                nc.vector.tensor_copy(nmgr[:], nmgT_p[:])
                nmgb_p = ps.tile([N_IN, H], F32, tag="small", bufs=3, name="nmgb_p")
                nc.tensor.matmul(nmgb_p[:], ones[:, 0:64], nmgr[:])
                da1 = wk.tile([N_IN, H], F32, tag="da1")
                nc.vector.tensor_sub(da1[:], n1[:], nmgb_p[:])
                al1 = wk.tile([N_IN, H], F32, tag="al1")
                nc.scalar.activation(al1[:], da1[:], AF.Exp, scale=-1.0)
                Vh1 = wk.tile([N_IN, HD], F32, tag="vh1")
                for h in range(H):
                    nc.vector.tensor_scalar_mul(
                        Vh1[:, 64 * h:64 * h + 64],
                        V[2][0:64, 64 * h:64 * h + 64], al1[:, h:h + 1])
                A1p = [ps.tile([128, J], F32, tag="small", bufs=3, name=f"a1_p{i}") for i in range(2)]
                for h in range(H):
                    nc.tensor.matmul(
                        A1p[h // 2][64 * (h % 2):64 * (h % 2) + 64, :],
                        Vh1[:, 64 * h:64 * h + 64], ULA[:, J * h:J * (h + 1)])
                A1s = [wk.tile([128, J], F32, tag=f"a1s{i}", name=f"a1s{i}") for i in range(2)]
                nc.vector.tensor_copy(A1s[0][:], A1p[0][:])
                nc.vector.tensor_copy(A1s[1][:], A1p[1][:])

                if li == 0 and stage == "agg":
                    nc.sync.dma_start(out=xoutT_d[0:64, 0:256], in_=A1s[0][0:64, :])
                    nc.sync.dma_start(out=outvec_d[:], in_=A2G[0][0:64, :][:, 0:1])
                    return None

                # ---- output projection + residual + relu ----
                XCp = ps.tile([D, J], F32, tag="small", bufs=3, name="xc_p")
                nc.tensor.matmul(XCp[:], wos0[:], A1s[0][:], start=True, stop=False)
                nc.tensor.matmul(XCp[:], wos1[:], A1s[1][:], start=False, stop=True)
                XOp = ps.tile([D, N_OUT], F32, tag="small", bufs=3, name="xo_p")
                nc.tensor.matmul(XOp[:], wos0[:], A2G[0][:], start=True, stop=False)
                nc.tensor.matmul(XOp[:], wos1[:], A2G[1][:], start=False, stop=True)

                X2 = wk.tile([D, NB], F32, tag="x2")
                tC = wk.tile([D, J], F32, tag="tc_res")
                nc.vector.tensor_add(tC[:], XCp[:], XT[:, 0:J])
                nc.scalar.activation(X2[:, 0:J], tC[:], AF.Relu, bias=bo[:])
                tO = wk.tile([D, N_OUT], F32, tag="to_res")
                nc.vector.tensor_add(tO[:], XOp[:], XT[:, OCOL])
                nc.scalar.activation(X2[:, OCOL], tO[:], AF.Relu, bias=bo[:])
                nc.scalar.activation(X2[:, ICOL], XT[:, ICOL], AF.Relu, bias=bo[:])
                return X2

            X2 = layer(0, XT)
            if X2 is not None:
                X3 = layer(1, X2)

            if X2 is None:
                X3 = None
            if X3 is not None:
                # ---- final projection: sigmoid(x3[O] @ wproj + bproj) ----
                prj_p = ps.tile([N_OUT, 1], F32, tag="small", bufs=3, name="prj_p")
                nc.tensor.matmul(prj_p[:], X3[:, 320:384], WPRJ[:])
                bpb_p = ps.tile([N_OUT, 1], F32, tag="small", bufs=3, name="bpb_p")
                nc.tensor.matmul(bpb_p[:], ones[:, 0:64], BPRJ[:])
                bpb = wk.tile([N_OUT, 1], F32, tag="bpb")
                nc.vector.tensor_copy(bpb[:], bpb_p[:])
                ovec = wk.tile([N_OUT, 1], F32, tag="ovec")
                nc.scalar.activation(ovec[:], prj_p[:],
                                     mybir.ActivationFunctionType.Sigmoid,
                                     bias=bpb[:])
                nc.sync.dma_start(out=outvec_d[:], in_=ovec[:])
                nc.sync.dma_start(out=xoutT_d[:], in_=X3[:])

    nc.compile()
    return nc


# ---------------------------------------------------------------------------
# Host-side sharding / runner
# ---------------------------------------------------------------------------

def _shard_inputs(inputs):
    """Build the per-core input maps for the Bass program."""
    x = np.asarray(inputs["node_features"], np.float32).copy()
    x[:N_IN, 0] = np.asarray(inputs["x_input"], np.float32)
    xT = np.ascontiguousarray(x.T)  # [64, 2176]
    ew = np.asarray(inputs["edge_weights"], np.float32).reshape(-1)
    ew1b = ew[:E1].reshape(N_IN, N_HID)
    ew2b = ew[E1:].reshape(N_HID, N_OUT)

    base = {}
    for l in (1, 2):
        base[f"wq{l}"] = np.ascontiguousarray(np.asarray(inputs[f"wq{l}"], np.float32))
        base[f"wks{l}"] = np.ascontiguousarray(
            np.asarray(inputs[f"wk{l}"], np.float32) * 0.125)
        base[f"wv{l}"] = np.ascontiguousarray(np.asarray(inputs[f"wv{l}"], np.float32))
        base[f"wo{l}"] = np.ascontiguousarray(np.asarray(inputs[f"wo{l}"], np.float32))
        base[f"we{l}"] = np.ascontiguousarray(np.asarray(inputs[f"we{l}"], np.float32))
        base[f"bo{l}"] = np.ascontiguousarray(
            np.asarray(inputs[f"bo{l}"], np.float32).reshape(D, 1))
    base["wproj"] = np.ascontiguousarray(np.asarray(inputs["wproj"], np.float32))
    base["bproj"] = np.ascontiguousarray(
        np.asarray(inputs["bproj"], np.float32).reshape(1, 1))

    in_maps = []
    for c in range(NCORES):
        j0 = J * c
        cols = np.concatenate([
            np.arange(N_IN + j0, N_IN + j0 + J),       # chunk hidden nodes
            np.arange(0, N_IN),                         # input nodes
            np.arange(N_IN + N_HID, N_NODES),           # output nodes
        ])
        m = dict(base)
        m["xT"] = np.ascontiguousarray(xT[:, cols])
        m["ew1"] = np.ascontiguousarray(ew1b[:, j0:j0 + J])
        ew2c = ew2b[j0:j0 + J, :]  # [256, 64]
        m["ew2p"] = np.ascontiguousarray(
            ew2c.reshape(2, 128, N_OUT).transpose(1, 0, 2).reshape(128, 128))
        in_maps.append(m)
    return in_maps


def _unshard(results):
    x_full = np.empty((N_NODES, D), np.float32)
    r0 = results[0]["xoutT"]
    x_full[:N_IN] = r0[:, 256:320].T
    x_full[N_IN + N_HID:] = r0[:, 320:384].T
    for c in range(NCORES):
        x_full[N_IN + J * c:N_IN + J * (c + 1)] = results[c]["xoutT"][:, 0:J].T
    out = results[0]["outvec"].reshape(N_OUT).astype(np.float32)
    return out, x_full


@functools.lru_cache(maxsize=1)
def _get_runner():
    """Build + compile the program once; return a callable(in_maps) -> results."""
    import jax
    from jax.sharding import Mesh, PartitionSpec
    try:
        from jax.experimental.shard_map import shard_map
    except ImportError:
        from jax.experimental.maps import shard_map  # pragma: no cover
    import concourse.mybir as mybir
    from concourse.bass2jax import (_bass_exec_p, install_neuronx_cc_hook,
                                    partition_id_tensor)

    nc = _build_program()
    install_neuronx_cc_hook()

    partition_name = nc.partition_id_tensor.name if nc.partition_id_tensor else None
    in_names, out_names, out_avals, out_shapes = [], [], [], []
    for alloc in nc.m.functions[0].allocations:
        if not isinstance(alloc, mybir.MemoryLocationSet):
            continue
        name = alloc.memorylocations[0].name
        if alloc.kind == "ExternalInput":
            if name != partition_name:
                in_names.append(name)
        elif alloc.kind == "ExternalOutput":
            out_names.append(name)
            shape = tuple(alloc.tensor_shape)
            dtype = mybir.dt.np(alloc.dtype)
            out_avals.append(jax.core.ShapedArray(shape, dtype))
            out_shapes.append((shape, dtype))
    n_params = len(in_names)
    n_outs = len(out_avals)
    in_names_all = list(in_names) + out_names + (
        [partition_name] if partition_name else [])

    def _body(*args):
        operands = list(args)
        if partition_name is not None:
            operands.append(partition_id_tensor())
        outs = _bass_exec_p.bind(
            *operands, out_avals=tuple(out_avals), in_names=tuple(in_names_all),
            out_names=tuple(out_names), lowering_input_output_aliases=(),
            sim_require_finite=False, sim_require_nnan=False, nc=nc)
        return tuple(outs)

    devices = jax.devices()[:NCORES]
    mesh = Mesh(np.asarray(devices), ("core",))
    donate = tuple(range(n_params, n_params + n_outs))
    sharded = jax.jit(
        shard_map(_body, mesh=mesh,
                  in_specs=(PartitionSpec("core"),) * (n_params + n_outs),
                  out_specs=(PartitionSpec("core"),) * n_outs,
                  check_rep=False),
        donate_argnums=donate, keep_unused=True)

    def run(in_maps):
        concat_in = [
            np.concatenate([in_maps[c][name] for c in range(NCORES)], axis=0)
            for name in in_names]
        zeros = [np.zeros((NCORES * s[0], *s[1:]), dt) for (s, dt) in out_shapes]
        out_arrs = sharded(*concat_in, *zeros)
        return [
            {name: np.asarray(out_arrs[i]).reshape(NCORES, *out_shapes[i][0])[c]
             for i, name in enumerate(out_names)}
            for c in range(NCORES)]

    return run


# ---------------------------------------------------------------------------
# General fallback (arbitrary edge_index) — exact reference math via jax
# ---------------------------------------------------------------------------

def _fallback(inputs):
    import jax
    import jax.numpy as jnp

    def message_pass(x, edge_index, ew, wq, wk, wv, we, wo, bo):
        src, tgt = edge_index[0], edge_index[1]
        dout = wo.shape[1]
        heads = wq.shape[1] // dout
        q = (x[tgt] @ wq).reshape(-1, heads, dout)
        k = (x[src] @ wk).reshape(-1, heads, dout)
        v = (x[src] @ wv).reshape(-1, heads, dout)
        attn = (q * k).sum(-1) / jnp.sqrt(jnp.float32(dout)) + ew @ we
        attn = jax.nn.leaky_relu(attn, negative_slope=0.2)
        attn = jax.nn.softmax(attn, axis=0)
        weighted_v = (attn[:, :, None] * v).reshape(-1, heads * dout)
        out = jax.ops.segment_sum(weighted_v, tgt, num_segments=x.shape[0])
        return out @ wo + bo + x

    f = inputs
    x = jnp.asarray(f["node_features"], jnp.float32)
    x = x.at[:jnp.asarray(f["x_input"]).shape[0], 0].set(jnp.asarray(f["x_input"]))
    ei = jnp.asarray(f["edge_index"], jnp.int32)
    ew = jnp.asarray(f["edge_weights"], jnp.float32)
    x = jax.nn.relu(message_pass(x, ei, ew, f["wq1"], f["wk1"], f["wv1"],
                                 f["we1"], f["wo1"], f["bo1"]))
    x = jax.nn.relu(message_pass(x, ei, ew, f["wq2"], f["wk2"], f["wv2"],
                                 f["we2"], f["wo2"], f["bo2"]))
    n_out = int(f["num_output_nodes"])
    out_nodes = x[x.shape[0] - n_out:]
    output = jax.nn.sigmoid(out_nodes @ jnp.asarray(f["wproj"])
                            + jnp.asarray(f["bproj"])).squeeze()
    return np.asarray(output, np.float32), np.asarray(x, np.float32)


# ---------------------------------------------------------------------------

def _is_structured(inputs):
    try:
        ei = np.asarray(inputs["edge_index"])
        if ei.shape != (2, E):
            return False
        if int(inputs["num_output_nodes"]) != N_OUT:
            return False
        if np.asarray(inputs["node_features"]).shape != (N_NODES, D):
            return False
        return bool(np.array_equal(ei, _expected_edge_index()))
    except Exception:
        return False


def kernel(**inputs):
    if not _is_structured(inputs):
        return _fallback(inputs)
    run = _get_runner()
    results = run(_shard_inputs(inputs))
    return _unshard(results)


# revision 13
# speedup vs baseline: 1.8639x; 1.8639x over previous
"""Trainium2 Bass kernel for nn_DynamicGraphNet (2-layer GNN attention message passing).

Contract: kernel(**inputs) takes the FULL unsharded inputs (as produced by the
reference's setup_inputs) and returns the full output (output_vec[64], x[2176, 64]).

Strategy: the reference's edge_index is a deterministic complete-bipartite
block graph (64 input -> 2048 hidden -> 64 output).  We verify that structure
at runtime; when it holds, gather/scatter collapses into dense per-head
matmuls.  The 2048 hidden nodes are sharded across 8 NeuronCores (256 each);
the global edge-softmax and the output-node scatter-add are handled with one
small AllGather per layer (softmax max/sum stats + per-core partial
aggregates for the 64 output nodes).  If the structure check fails, a general
jax fallback implements the exact reference math for arbitrary graphs.
"""

import functools
import os
import sys

import numpy as np

for _p in ("/root/.axon_site", "/root/.axon_site/_ro/trn_rl_repo",
           "/root/.axon_site/_ro/pypackages", "/opt/trn_rl_repo"):
    if os.path.isdir(_p) and _p not in sys.path:
        sys.path.append(_p)

N_NODES = 2176
N_IN = 64
N_HID = 2048
N_OUT = 64
D = 64
H = 4
HD = D * H  # 256
E1 = N_IN * N_HID
E2 = N_HID * N_OUT
E = E1 + E2
NCORES = 8
J = N_HID // NCORES  # 256 hidden nodes per core
NB = 384  # nodes held per core: J chunk + N_IN + N_OUT
PAY = 8 + 2 * 128 * 64  # collective payload floats per core: stats + 2 agg tiles
# packed input blob layout (floats)
OFF_XT = 0
OFF_EW1 = D * NB            # 24576
OFF_EW2P = OFF_EW1 + N_IN * J   # 40960
NS_BLOB = OFF_EW2P + 128 * 128  # 57344
LAYER_W = 4 * 16384 + H + D     # wq+wks+wv+wo then we, bo = 65604
NW_BLOB = 2 * LAYER_W + D + 1   # + wproj + bproj


def _expected_edge_index() -> np.ndarray:
    hs, os_ = N_IN, N_IN + N_HID
    s1 = np.repeat(np.arange(0, N_IN), N_HID)
    t1 = np.tile(np.arange(hs, os_), N_IN)
    s2 = np.repeat(np.arange(hs, os_), N_OUT)
    t2 = np.tile(np.arange(os_, N_NODES), N_HID)
    return np.stack([np.concatenate([s1, s2]), np.concatenate([t1, t2])]).astype(np.int32)


# ---------------------------------------------------------------------------
# Bass program
# ---------------------------------------------------------------------------

def _build_program():
    import concourse.bacc as bacc
    import concourse.mybir as mybir
    from concourse import masks
    from concourse.tile import TileContext

    F32 = mybir.dt.float32
    AF = mybir.ActivationFunctionType
    ALU = mybir.AluOpType
    AX = mybir.AxisListType

    nc = bacc.Bacc("TRN2", target_bir_lowering=False, debug=False,
                   num_devices=NCORES)

    # ---- I/O (packed blobs to minimise per-call H2D round trips) ----
    xsh_d = nc.dram_tensor("xsh", [1, NS_BLOB], F32, kind="ExternalInput")
    wrepl_d = nc.dram_tensor("wrepl", [1, NW_BLOB], F32, kind="ExternalInput")
    xout_d = nc.dram_tensor("xout", [D, NB + 1], F32, kind="ExternalOutput")
    xoutT_d = xout_d[:, 0:NB]
    outvec_d = xout_d[:, NB:NB + 1]

    def sh_view(off, p, f):
        return xsh_d[0:1, off:off + p * f].rearrange("a (p f) -> (a p) f", p=p)

    def w_view(off, p, f):
        return wrepl_d[0:1, off:off + p * f].rearrange("a (p f) -> (a p) f", p=p)

    with TileContext(nc) as tc:
        with (
            tc.tile_pool(name="consts", bufs=1) as cp,
            tc.tile_pool(name="wts", bufs=1) as wpool,
            tc.tile_pool(name="work", bufs=2) as wk,
            tc.tile_pool(name="ps", bufs=1, space="PSUM") as ps,
            tc.tile_pool(name="dram", bufs=1, space="DRAM") as dr,
        ):
            # ---- constants ----
            ident = cp.tile([128, 128], F32)
            masks.make_identity(nc, ident[:])
            ones = cp.tile([1, 128], F32)
            nc.vector.memset(ones[:], 1.0)
            # head-expansion one-hots: expA[k, m]=1 iff m//64==k (k<2); expB for k-2
            expA_cols = cp.tile([128, H], F32)
            expB_cols = cp.tile([128, H], F32)
            nc.vector.memset(expA_cols[:], 0.0)
            nc.vector.memset(expB_cols[:], 0.0)
            nc.vector.memset(expA_cols[0:64, 0:1], 1.0)
            nc.vector.memset(expA_cols[64:128, 1:2], 1.0)
            nc.vector.memset(expB_cols[0:64, 2:3], 1.0)
            nc.vector.memset(expB_cols[64:128, 3:4], 1.0)

            expA_p = ps.tile([H, 128], F32, tag="small", bufs=3, name="expA_p")
            expB_p = ps.tile([H, 128], F32, tag="small", bufs=3, name="expB_p")
            nc.tensor.transpose(expA_p[:], expA_cols[:], ident[:])
            nc.tensor.transpose(expB_p[:], expB_cols[:], ident[:])
            expA = cp.tile([H, 128], F32)
            expB = cp.tile([H, 128], F32)
            nc.vector.tensor_copy(expA[:], expA_p[:])
            nc.vector.tensor_copy(expB[:], expB_p[:])

            # ---- load inputs from packed blobs ----
            def load_from(view, shape, name):
                t = wpool.tile(shape, F32, tag=name, name=name)
                nc.sync.dma_start(out=t[:], in_=view)
                return t

            XT = load_from(sh_view(OFF_XT, D, NB), [D, NB], "xT")
            EW1 = load_from(sh_view(OFF_EW1, N_IN, J), [N_IN, J], "ew1")
            EW2P = load_from(sh_view(OFF_EW2P, 128, 128), [128, 128], "ew2p")
            WQ, WKS, WV, WE, BO, WO0, WO1 = [], [], [], [], [], [], []
            for i in range(2):
                base = i * LAYER_W
                WQ.append(load_from(w_view(base, D, HD), [D, HD], f"wq{i}"))
                WKS.append(load_from(w_view(base + 16384, D, HD), [D, HD], f"wks{i}"))
                WV.append(load_from(w_view(base + 32768, D, HD), [D, HD], f"wv{i}"))
                WO0.append(load_from(w_view(base + 49152, 128, D), [128, D], f"wo{i}a"))
                WO1.append(load_from(w_view(base + 49152 + 8192, 128, D), [128, D], f"wo{i}b"))
                WE.append(load_from(w_view(base + 65536, 1, H), [1, H], f"we{i}"))
                BO.append(load_from(w_view(base + 65540, D, 1), [D, 1], f"bo{i}"))
            WPRJ = load_from(w_view(2 * LAYER_W, D, 1), [D, 1], "wproj")
            BPRJ = load_from(w_view(2 * LAYER_W + D, 1, 1), [1, 1], "bproj")

            # collective bounce buffers (per layer)
            ccin = [dr.tile([1, PAY], F32, tag=f"ccin{l}", name=f"ccin{l}") for l in range(2)]
            ccout = [dr.tile([NCORES, PAY], F32, tag=f"ccout{l}", name=f"ccout{l}") for l in range(2)]

            stage = os.environ.get("GNN_STAGE", "full")

            def layer(li, XT):
                """One message-passing layer; returns new XT [D, NB] tile."""
                wq, wks, wv, we, bo = WQ[li], WKS[li], WV[li], WE[li], BO[li]
                wo0, wo1 = WO0[li], WO1[li]

                # ---- projections ----
                # Per-head QT/KT [64, NB] so every matmul operand sits at
                # base partition 0 (HW rejects mixed operand bases within a
                # PSUM tile's accumulation groups).
                QThp = [ps.tile([D, NB], F32, tag="qkv", bufs=2, name=f"qt_p{h}")
                        for h in range(H)]
                KThp = [ps.tile([D, NB], F32, tag="qkv", bufs=2, name=f"kt_p{h}")
                        for h in range(H)]
                for h in range(H):
                    nc.tensor.matmul(QThp[h][:], wq[:, 64 * h:64 * h + 64], XT[:])
                    nc.tensor.matmul(KThp[h][:], wks[:, 64 * h:64 * h + 64], XT[:])
                # V node-major [NB, HD] (3 partition tiles: chunk0, chunk1, I+O)
                Vp = [ps.tile([128, NB], F32, tag="qkv", bufs=2, name=f"v_p{t}")[:, 0:HD] for t in range(3)]
                for t in range(3):
                    nc.tensor.matmul(Vp[t][:], XT[:, 128 * t:128 * (t + 1)], wv[:])
                QTh = [wk.tile([D, NB], F32, tag=f"qth{h}", name=f"qth{h}") for h in range(H)]
                KTh = [wk.tile([D, NB], F32, tag=f"kth{h}", name=f"kth{h}") for h in range(H)]
                V = [wk.tile([128, HD], F32, tag=f"v{t}", name=f"v{t}") for t in range(3)]
                for h in range(H):
                    nc.vector.tensor_copy(QTh[h][:], QThp[h][:])
                    nc.scalar.copy(KTh[h][:], KThp[h][:])
                for t in range(3):
                    nc.vector.tensor_copy(V[t][:], Vp[t][:])

                if li == 0 and stage == "proj":
                    nc.sync.dma_start(out=xoutT_d[:], in_=QTh[0][:, :])
                    nc.sync.dma_start(out=outvec_d[:], in_=V[0][0:64, 0:1])
                    return None

                def qt_h(h, cols):
                    return QTh[h][:, cols]

                def kt_h(h, cols):
                    return KTh[h][:, cols]

                CH0, CH1 = slice(0, 128), slice(128, 256)
                ICOL, OCOL = slice(256, 320), slice(320, 384)

                # ---- logits ----
                # S1[i, h*J + j] = (K_h[I] . Q_h[chunk_j]) / 8
                S1p = ps.tile([N_IN, H * J], F32, tag="s1", name="s1_p")
                for h in range(H):
                    nc.tensor.matmul(S1p[:, J * h:J * (h + 1)],
                                     kt_h(h, ICOL), qt_h(h, slice(0, J)))
                # S2[p, jh*256 + h*64 + o] = (K_h[chunk_{jh*128+p}] . Q_h[O_o]) / 8
                S2p = ps.tile([128, 512], F32, tag="s2", name="s2_p")
                for h in range(H):
                    for jh in range(2):
                        nc.tensor.matmul(
                            S2p[:, 256 * jh + 64 * h:256 * jh + 64 * h + 64],
                            kt_h(h, (CH0, CH1)[jh]), qt_h(h, OCOL))

                if li == 0 and stage == "s1s2":
                    o1 = wk.tile([64, 512], F32, tag="dbg1", name="dbg1")
                    nc.vector.tensor_copy(o1[:], S1p[:, 0:512])
                    o2 = wk.tile([128, 384], F32, tag="dbg2", name="dbg2")
                    nc.vector.tensor_copy(o2[:], S2p[:, 0:384])
                    nc.sync.dma_start(out=xoutT_d[:], in_=o1[0:64, 0:384])
                    nc.sync.dma_start(out=outvec_d[:], in_=o2[0:64, 0:1])
                    return None

                # ---- edge-weight term + add + leaky relu ----
                web_p = ps.tile([128, H], F32, tag="small", bufs=3, name="web_p")
                nc.tensor.matmul(web_p[:], ones[:], we[:])
                web = wk.tile([128, H], F32, tag="web")
                nc.vector.tensor_copy(web[:], web_p[:])

                LA = wk.tile([N_IN, H * J], F32, tag="la")
                LB = wk.tile([128, 512], F32, tag="lb")
                for h in range(H):
                    nc.vector.tensor_scalar_mul(
                        LA[:, J * h:J * (h + 1)], EW1[:], web[0:64, h:h + 1])
                for h in range(H):
                    for jh in range(2):
                        nc.vector.tensor_scalar_mul(
                            LB[:, 256 * jh + 64 * h:256 * jh + 64 * h + 64],
                            EW2P[:, 64 * jh:64 * jh + 64], web[:, h:h + 1])
                nc.vector.tensor_add(LA[:], LA[:], S1p[:])
                nc.vector.tensor_add(LB[:], LB[:], S2p[:])
                if li == 0 and stage == "eww":
                    nc.sync.dma_start(out=xoutT_d[:], in_=LA[0:64, 0:NB])
                    nc.sync.dma_start(out=outvec_d[:], in_=LB[0:64, 0:1])
                    return None
                # leaky_relu(x) = max(x, 0.2*x)
                LAt = wk.tile([N_IN, H * J], F32, tag="lat")
                LBt = wk.tile([128, 512], F32, tag="lbt")
                nc.scalar.activation(LAt[:], LA[:], AF.Identity, scale=0.2)
                nc.scalar.activation(LBt[:], LB[:], AF.Identity, scale=0.2)
                nc.vector.tensor_max(LA[:], LA[:], LAt[:])
                nc.vector.tensor_max(LB[:], LB[:], LBt[:])

                if li == 0 and stage == "lrelu":
                    nc.sync.dma_start(out=xoutT_d[:], in_=LA[0:64, 0:NB])
                    nc.sync.dma_start(out=outvec_d[:], in_=LB[0:64, 0:1])
                    return None
                # ---- per-partition (negated) maxes ----
                n1 = wk.tile([N_IN, H], F32, tag="n1")
                for h in range(H):
                    nc.vector.tensor_reduce(
                        out=n1[:, h:h + 1], in_=LA[:, J * h:J * (h + 1)],
                        op=ALU.max, axis=AX.X, negate=True)
                n2 = wk.tile([128, 2 * H], F32, tag="n2")
                for h in range(H):
                    for jh in range(2):
                        nc.vector.tensor_reduce(
                            out=n2[:, 4 * jh + h:4 * jh + h + 1],
                            in_=LB[:, 256 * jh + 64 * h:256 * jh + 64 * h + 64],
                            op=ALU.max, axis=AX.X, negate=True)

                if li == 0 and stage == "rowmax":
                    nc.sync.dma_start(out=xoutT_d[0:64, 0:4], in_=n1[:])
                    nc.sync.dma_start(out=outvec_d[0:64, 0:1], in_=n2[0:64, 0:1])
                    return None
                # ---- U = exp(L - rowmax), rowsums ----
                ULA = wk.tile([N_IN, H * J], F32, tag="ula")
                ULB = wk.tile([128, 512], F32, tag="ulb")
                rs1 = wk.tile([N_IN, H], F32, tag="rs1")
                rs2 = wk.tile([128, 2 * H], F32, tag="rs2")
                for h in range(H):
                    nc.scalar.activation(
                        ULA[:, J * h:J * (h + 1)], LA[:, J * h:J * (h + 1)],
                        AF.Exp, bias=n1[:, h:h + 1], accum_out=rs1[:, h:h + 1])
                for h in range(H):
                    for jh in range(2):
                        s = slice(256 * jh + 64 * h, 256 * jh + 64 * h + 64)
                        nc.scalar.activation(
                            ULB[:, s], LB[:, s], AF.Exp,
                            bias=n2[:, 4 * jh + h:4 * jh + h + 1],
                            accum_out=rs2[:, 4 * jh + h:4 * jh + h + 1])

                if li == 0 and stage == "exp":
                    nc.sync.dma_start(out=xoutT_d[:], in_=ULA[0:64, 0:NB])
                    nc.sync.dma_start(out=outvec_d[:], in_=rs2[0:64, 0:1])
                    return None

                # ---- core-local softmax stats (negated-max domain) ----
                # nm_c[h] = min(n1, n2) over partitions; s_c from rowsums
                n2m = wk.tile([128, H], F32, tag="n2m")
                nc.vector.tensor_tensor(n2m[:], n2[:, 0:4], n2[:, 4:8], op=ALU.min)
                n1T_p = ps.tile([H, N_IN], F32, tag="small", bufs=3, name="n1T_p")
                nc.tensor.transpose(n1T_p[:], n1[:], ident[0:64, 0:64])
                n2mT_p = ps.tile([H, 128], F32, tag="small", bufs=3, name="n2mT_p")
                nc.tensor.transpose(n2mT_p[:], n2m[:], ident[:])
                r1 = wk.tile([H, 1], F32, tag="r1")
                r2 = wk.tile([H, 1], F32, tag="r2")
                nc.vector.tensor_reduce(out=r1[:], in_=n1T_p[:], op=ALU.min, axis=AX.X)
                nc.vector.tensor_reduce(out=r2[:], in_=n2mT_p[:], op=ALU.min, axis=AX.X)
                nmc = wk.tile([H, 1], F32, tag="nmc")
                nc.vector.tensor_tensor(nmc[:], r1[:], r2[:], op=ALU.min)

                # s_c = sum(rowsum * exp(rowmax - m_c))
                rs1T_p = ps.tile([H, N_IN], F32, tag="small", bufs=3, name="rs1T_p")
                nc.tensor.transpose(rs1T_p[:], rs1[:], ident[0:64, 0:64])
                n1T_e = wk.tile([H, N_IN], F32, tag="n1te")
                nc.scalar.activation(n1T_e[:], n1T_p[:], AF.Exp,
                                     bias=nmc[:], scale=-1.0)
                pr1 = wk.tile([H, N_IN], F32, tag="pr1")
                nc.vector.tensor_mul(pr1[:], n1T_e[:], rs1T_p[:])
                sc = wk.tile([H, 1], F32, tag="sc")
                nc.vector.reduce_sum(out=sc[:], in_=pr1[:], axis=AX.X)
                for jh in range(2):
                    n2T_p = ps.tile([H, 128], F32, tag="small", bufs=3, name="n2T_p")
                    rs2T_p = ps.tile([H, 128], F32, tag="small", bufs=3, name="rs2T_p")
                    nc.tensor.transpose(n2T_p[:], n2[:, 4 * jh:4 * jh + 4], ident[:])
                    nc.tensor.transpose(rs2T_p[:], rs2[:, 4 * jh:4 * jh + 4], ident[:])
                    n2T_e = wk.tile([H, 128], F32, tag="n2te")
                    nc.scalar.activation(n2T_e[:], n2T_p[:], AF.Exp,
                                         bias=nmc[:], scale=-1.0)
                    pr2 = wk.tile([H, 128], F32, tag="pr2")
                    nc.vector.tensor_mul(pr2[:], n2T_e[:], rs2T_p[:])
                    sc2 = wk.tile([H, 1], F32, tag="sc2")
                    nc.vector.reduce_sum(out=sc2[:], in_=pr2[:], axis=AX.X)
                    nc.vector.tensor_add(sc[:], sc[:], sc2[:])

                # ---- V-hat for block 2 (scale V chunk rows by exp(rowmax - m_c)) ----
                nmcT_p = ps.tile([1, H], F32, tag="small", bufs=3, name="nmcT_p")
                nc.tensor.transpose(nmcT_p[:], nmc[:], ident[0:4, 0:4])
                row8 = wk.tile([1, 2 * H], F32, tag="row8")
                nc.vector.tensor_copy(row8[:, 0:4], nmcT_p[:])
                nc.vector.tensor_copy(row8[:, 4:8], nmcT_p[:])
                nmc8b_p = ps.tile([128, 2 * H], F32, tag="small", bufs=3, name="nmc8b_p")
                nc.tensor.matmul(nmc8b_p[:], ones[:], row8[:])
                dal2 = wk.tile([128, 2 * H], F32, tag="dal2")
                nc.vector.tensor_sub(dal2[:], n2[:], nmc8b_p[:])
                al2 = wk.tile([128, 2 * H], F32, tag="al2")
                nc.scalar.activation(al2[:], dal2[:], AF.Exp, scale=-1.0)
                Vh2 = [wk.tile([128, HD], F32, tag=f"vh2_{t}", name=f"vh2_{t}") for t in range(2)]
                for jh in range(2):
                    for h in range(H):
                        nc.vector.tensor_scalar_mul(
                            Vh2[jh][:, 64 * h:64 * h + 64],
                            V[jh][:, 64 * h:64 * h + 64],
                            al2[:, 4 * jh + h:4 * jh + h + 1])

                # ---- block-2 partial aggregation: AGG2T_h[d, o] ----
                A2p = [ps.tile([128, N_OUT], F32, tag="small", bufs=3, name=f"a2_p{i}") for i in range(2)]
                for h in range(H):
                    for jh in range(2):
                        nc.tensor.matmul(
                            A2p[h // 2][64 * (h % 2):64 * (h % 2) + 64, :],
                            Vh2[jh][:, 64 * h:64 * h + 64],
                            ULB[:, 256 * jh + 64 * h:256 * jh + 64 * h + 64],
                            start=(jh == 0), stop=(jh == 1))
                A2s = [wk.tile([128, N_OUT], F32, tag=f"a2s{i}", name=f"a2s{i}") for i in range(2)]
                nc.vector.tensor_copy(A2s[0][:], A2p[0][:])
                nc.vector.tensor_copy(A2s[1][:], A2p[1][:])

                # ---- payload DMA + AllGather ----
                ci, co = ccin[li], ccout[li]
                nc.sync.dma_start(out=ci[0:1, 0:4].rearrange("a b -> b a"), in_=nmc[:])
                nc.sync.dma_start(out=ci[0:1, 4:8].rearrange("a b -> b a"), in_=sc[:])
                nc.sync.dma_start(
                    out=ci[0:1, 8:8200].rearrange("a (p f) -> (a p) f", p=128),
                    in_=A2s[0][:])
                nc.sync.dma_start(
                    out=ci[0:1, 8200:16392].rearrange("a (p f) -> (a p) f", p=128),
                    in_=A2s[1][:])
                if li == 0 and stage == "payload":
                    nc.sync.dma_start(out=xoutT_d[:], in_=ULB[0:64, 0:NB])
                    nc.sync.dma_start(out=outvec_d[0:4, 0:1], in_=sc[:])
                    return None
                nc.gpsimd.collective_compute(
                    "AllGather", mybir.AluOpType.bypass,
                    replica_groups=[list(range(NCORES))],
                    ins=[ci.opt()], outs=[co.opt()])

                # ---- global stats ----
                Gm = wk.tile([NCORES, H], F32, tag="gm")
                Gs = wk.tile([NCORES, H], F32, tag="gs")
                nc.sync.dma_start(out=Gm[:], in_=co[:, 0:4])
                nc.sync.dma_start(out=Gs[:], in_=co[:, 4:8])
                GmT_p = ps.tile([H, NCORES], F32, tag="small", bufs=3, name="GmT_p")
                GsT_p = ps.tile([H, NCORES], F32, tag="small", bufs=3, name="GsT_p")
                nc.tensor.transpose(GmT_p[:], Gm[:], ident[0:8, 0:8])
                nc.tensor.transpose(GsT_p[:], Gs[:], ident[0:8, 0:8])
                nmg = wk.tile([H, 1], F32, tag="nmg")
                nc.vector.tensor_reduce(out=nmg[:], in_=GmT_p[:], op=ALU.min, axis=AX.X)
                # w[h, c] = exp(m_c - m_g)
                wc = wk.tile([H, NCORES], F32, tag="wc")
                nc.scalar.activation(wc[:], GmT_p[:], AF.Exp, bias=nmg[:], scale=-1.0)
                spr = wk.tile([H, NCORES], F32, tag="spr")
                nc.vector.tensor_mul(spr[:], wc[:], GsT_p[:])
                Sg = wk.tile([H, 1], F32, tag="sg")
                nc.vector.reduce_sum(out=Sg[:], in_=spr[:], axis=AX.X)
                sinv = wk.tile([H, 1], F32, tag="sinv")
                nc.vector.reciprocal(sinv[:], Sg[:])

                if li == 0 and stage == "gather":
                    nc.sync.dma_start(out=xoutT_d[0:4, 0:8], in_=wc[:])
                    nc.sync.dma_start(out=outvec_d[0:4, 0:1], in_=Sg[:])
                    return None

                # ---- gather-weighted sum of block-2 partials ----
                wexp_p = [ps.tile([128, NCORES], F32, tag="small", bufs=3, name="wexp_p0"),
                          ps.tile([128, NCORES], F32, tag="small", bufs=3, name="wexp_p1")]
                nc.tensor.matmul(wexp_p[0][:], expA[:], wc[:])
                nc.tensor.matmul(wexp_p[1][:], expB[:], wc[:])
                wexp = [wk.tile([128, NCORES], F32, tag=f"wexp{i}", name=f"wexp{i}") for i in range(2)]
                nc.vector.tensor_copy(wexp[0][:], wexp_p[0][:])
                nc.vector.tensor_copy(wexp[1][:], wexp_p[1][:])
                A2G = [wk.tile([128, N_OUT], F32, tag=f"a2g{i}", name=f"a2g{i}") for i in range(2)]
                for i in range(2):
                    off = 8 + 8192 * i
                    for c in range(NCORES):
                        part = wk.tile([128, N_OUT], F32, tag=f"part{i}", name=f"part{i}")
                        nc.sync.dma_start(
                            out=part[:],
                            in_=co[c:c + 1, off:off + 8192].rearrange(
                                "a (p f) -> (a p) f", p=128))
                        if c == 0:
                            nc.vector.tensor_scalar_mul(
                                A2G[i][:], part[:], wexp[i][:, 0:1])
                        else:
                            tmp = wk.tile([128, N_OUT], F32, tag=f"tmp{i}", name=f"tmp{i}")
                            nc.vector.tensor_scalar_mul(
                                tmp[:], part[:], wexp[i][:, c:c + 1])
                            nc.vector.tensor_add(A2G[i][:], A2G[i][:], tmp[:])

                # ---- wo scaled by 1/S_g per head-block of rows ----
                rsc_p = [ps.tile([128, 1], F32, tag="small", bufs=3, name="rsc_p0"),
                         ps.tile([128, 1], F32, tag="small", bufs=3, name="rsc_p1")]
                nc.tensor.matmul(rsc_p[0][:], expA[:], sinv[:])
                nc.tensor.matmul(rsc_p[1][:], expB[:], sinv[:])
                rsc = [wk.tile([128, 1], F32, tag=f"rsc{i}", name=f"rsc{i}") for i in range(2)]
                nc.vector.tensor_copy(rsc[0][:], rsc_p[0][:])
                nc.vector.tensor_copy(rsc[1][:], rsc_p[1][:])
                wos0 = wk.tile([128, D], F32, tag="wos0")
                wos1 = wk.tile([128, D], F32, tag="wos1")
                nc.vector.tensor_scalar_mul(wos0[:], wo0[:], rsc[0][:])
                nc.vector.tensor_scalar_mul(wos1[:], wo1[:], rsc[1][:])

                # ---- block-1 aggregation (post-gather; fold exp(rowmax-m_g) into V_I) ----
                nmgT_p = ps.tile([1, H], F32, tag="small", bufs=3, name="nmgT_p")
                nc.tensor.transpose(nmgT_p[:], nmg[:], ident[0:4, 0:4])
                nmgr = wk.tile([1, H], F32, tag="nmgr")
                nc.vector.tensor_copy(nmgr[:], nmgT_p[:])
                nmgb_p = ps.tile([N_IN, H], F32, tag="small", bufs=3, name="nmgb_p")
                nc.tensor.matmul(nmgb_p[:], ones[:, 0:64], nmgr[:])
                da1 = wk.tile([N_IN, H], F32, tag="da1")
                nc.vector.tensor_sub(da1[:], n1[:], nmgb_p[:])
                al1 = wk.tile([N_IN, H], F32, tag="al1")
                nc.scalar.activation(al1[:], da1[:], AF.Exp, scale=-1.0)
                Vh1 = wk.tile([N_IN, HD], F32, tag="vh1")
                for h in range(H):
                    nc.vector.tensor_scalar_mul(
                        Vh1[:, 64 * h:64 * h + 64],
                        V[2][0:64, 64 * h:64 * h + 64], al1[:, h:h + 1])
                A1p = [ps.tile([128, J], F32, tag="small", bufs=3, name=f"a1_p{i}") for i in range(2)]
                for h in range(H):
                    nc.tensor.matmul(
                        A1p[h // 2][64 * (h % 2):64 * (h % 2) + 64, :],
                        Vh1[:, 64 * h:64 * h + 64], ULA[:, J * h:J * (h + 1)])
                A1s = [wk.tile([128, J], F32, tag=f"a1s{i}", name=f"a1s{i}") for i in range(2)]
                nc.vector.tensor_copy(A1s[0][:], A1p[0][:])
                nc.vector.tensor_copy(A1s[1][:], A1p[1][:])

                if li == 0 and stage == "agg":
                    nc.sync.dma_start(out=xoutT_d[0:64, 0:256], in_=A1s[0][0:64, :])
                    nc.sync.dma_start(out=outvec_d[:], in_=A2G[0][0:64, :][:, 0:1])
                    return None

                # ---- output projection + residual + relu ----
                XCp = ps.tile([D, J], F32, tag="small", bufs=3, name="xc_p")
                nc.tensor.matmul(XCp[:], wos0[:], A1s[0][:], start=True, stop=False)
                nc.tensor.matmul(XCp[:], wos1[:], A1s[1][:], start=False, stop=True)
                XOp = ps.tile([D, N_OUT], F32, tag="small", bufs=3, name="xo_p")
                nc.tensor.matmul(XOp[:], wos0[:], A2G[0][:], start=True, stop=False)
                nc.tensor.matmul(XOp[:], wos1[:], A2G[1][:], start=False, stop=True)

                X2 = wk.tile([D, NB], F32, tag="x2")
                tC = wk.tile([D, J], F32, tag="tc_res")
                nc.vector.tensor_add(tC[:], XCp[:], XT[:, 0:J])
                nc.scalar.activation(X2[:, 0:J], tC[:], AF.Relu, bias=bo[:])
                tO = wk.tile([D, N_OUT], F32, tag="to_res")
                nc.vector.tensor_add(tO[:], XOp[:], XT[:, OCOL])
                nc.scalar.activation(X2[:, OCOL], tO[:], AF.Relu, bias=bo[:])
                nc.scalar.activation(X2[:, ICOL], XT[:, ICOL], AF.Relu, bias=bo[:])
                return X2

            X2 = layer(0, XT)
            if X2 is not None:
                X3 = layer(1, X2)

            if X2 is None:
                X3 = None
            if X3 is not None:
                # ---- final projection: sigmoid(x3[O] @ wproj + bproj) ----
                prj_p = ps.tile([N_OUT, 1], F32, tag="small", bufs=3, name="prj_p")
                nc.tensor.matmul(prj_p[:], X3[:, 320:384], WPRJ[:])
                bpb_p = ps.tile([N_OUT, 1], F32, tag="small", bufs=3, name="bpb_p")
                nc.tensor.matmul(bpb_p[:], ones[:, 0:64], BPRJ[:])
                bpb = wk.tile([N_OUT, 1], F32, tag="bpb")
                nc.vector.tensor_copy(bpb[:], bpb_p[:])
                ovec = wk.tile([N_OUT, 1], F32, tag="ovec")
                nc.scalar.activation(ovec[:], prj_p[:],
                                     mybir.ActivationFunctionType.Sigmoid,
                                     bias=bpb[:])
                nc.sync.dma_start(out=outvec_d[:], in_=ovec[:])
                nc.sync.dma_start(out=xoutT_d[:], in_=X3[:])

    nc.compile()
    return nc


# ---------------------------------------------------------------------------
# Host-side sharding / runner
# ---------------------------------------------------------------------------

def _shard_inputs(inputs):
    """Build the per-core input maps for the Bass program."""
    x = np.asarray(inputs["node_features"], np.float32).copy()
    x[:N_IN, 0] = np.asarray(inputs["x_input"], np.float32)
    xT = np.ascontiguousarray(x.T)  # [64, 2176]
    ew = np.asarray(inputs["edge_weights"], np.float32).reshape(-1)
    ew1b = ew[:E1].reshape(N_IN, N_HID)
    ew2b = ew[E1:].reshape(N_HID, N_OUT)

    base = {}
    for l in (1, 2):
        base[f"wq{l}"] = np.ascontiguousarray(np.asarray(inputs[f"wq{l}"], np.float32))
        base[f"wks{l}"] = np.ascontiguousarray(
            np.asarray(inputs[f"wk{l}"], np.float32) * 0.125)
        base[f"wv{l}"] = np.ascontiguousarray(np.asarray(inputs[f"wv{l}"], np.float32))
        base[f"wo{l}"] = np.ascontiguousarray(np.asarray(inputs[f"wo{l}"], np.float32))
        base[f"we{l}"] = np.ascontiguousarray(np.asarray(inputs[f"we{l}"], np.float32))
        base[f"bo{l}"] = np.ascontiguousarray(
            np.asarray(inputs[f"bo{l}"], np.float32).reshape(D, 1))
    base["wproj"] = np.ascontiguousarray(np.asarray(inputs["wproj"], np.float32))
    base["bproj"] = np.ascontiguousarray(
        np.asarray(inputs["bproj"], np.float32).reshape(1, 1))

    wrepl = np.concatenate([
        np.concatenate([base[f"wq{l}"].ravel(), base[f"wks{l}"].ravel(),
                        base[f"wv{l}"].ravel(), base[f"wo{l}"].ravel(),
                        base[f"we{l}"].ravel(), base[f"bo{l}"].ravel()])
        for l in (1, 2)
    ] + [base["wproj"].ravel(), base["bproj"].ravel()])[None, :].astype(np.float32)
    assert wrepl.shape[1] == NW_BLOB

    shards = []
    for c in range(NCORES):
        j0 = J * c
        cols = np.concatenate([
            np.arange(N_IN + j0, N_IN + j0 + J),       # chunk hidden nodes
            np.arange(0, N_IN),                         # input nodes
            np.arange(N_IN + N_HID, N_NODES),           # output nodes
        ])
        ew2c = ew2b[j0:j0 + J, :]  # [256, 64]
        ew2p = ew2c.reshape(2, 128, N_OUT).transpose(1, 0, 2).reshape(128, 128)
        blob = np.concatenate([
            np.ascontiguousarray(xT[:, cols]).ravel(),
            np.ascontiguousarray(ew1b[:, j0:j0 + J]).ravel(),
            ew2p.ravel()])[None, :].astype(np.float32)
        assert blob.shape[1] == NS_BLOB
        shards.append(blob)
    return shards, wrepl


def _unshard(results):
    # results: array [NCORES, D, NB + 1] (packed xoutT | outvec)
    x_full = np.empty((N_NODES, D), np.float32)
    r0 = results[0]
    x_full[:N_IN] = r0[:, 256:320].T
    x_full[N_IN + N_HID:] = r0[:, 320:384].T
    for c in range(NCORES):
        x_full[N_IN + J * c:N_IN + J * (c + 1)] = results[c][:, 0:J].T
    out = r0[:, NB].reshape(N_OUT).astype(np.float32)
    return out, x_full


@functools.lru_cache(maxsize=1)
def _get_runner():
    """Build + compile the program once; return a callable(shards, wrepl) -> results."""
    import jax
    from jax.sharding import Mesh, PartitionSpec
    try:
        from jax.experimental.shard_map import shard_map
    except ImportError:
        from jax.experimental.maps import shard_map  # pragma: no cover
    import concourse.mybir as mybir
    from concourse.bass2jax import (_bass_exec_p, install_neuronx_cc_hook,
                                    partition_id_tensor)

    nc = _build_program()
    install_neuronx_cc_hook()

    partition_name = nc.partition_id_tensor.name if nc.partition_id_tensor else None
    in_names, out_names, out_avals, out_shapes = [], [], [], []
    for alloc in nc.m.functions[0].allocations:
        if not isinstance(alloc, mybir.MemoryLocationSet):
            continue
        name = alloc.memorylocations[0].name
        if alloc.kind == "ExternalInput":
            if name != partition_name:
                in_names.append(name)
        elif alloc.kind == "ExternalOutput":
            out_names.append(name)
            shape = tuple(alloc.tensor_shape)
            dtype = mybir.dt.np(alloc.dtype)
            out_avals.append(jax.core.ShapedArray(shape, dtype))
            out_shapes.append((shape, dtype))
    assert sorted(in_names) == ["wrepl", "xsh"], in_names
    assert out_names == ["xout"], out_names
    n_params = len(in_names)
    n_outs = len(out_avals)
    in_names_all = list(in_names) + out_names + (
        [partition_name] if partition_name else [])

    def _body(*args):
        operands = list(args)
        if partition_name is not None:
            operands.append(partition_id_tensor())
        outs = _bass_exec_p.bind(
            *operands, out_avals=tuple(out_avals), in_names=tuple(in_names_all),
            out_names=tuple(out_names), lowering_input_output_aliases=(),
            sim_require_finite=False, sim_require_nnan=False, nc=nc)
        return tuple(outs)

    devices = jax.devices()[:NCORES]
    mesh = Mesh(np.asarray(devices), ("core",))
    donate = tuple(range(n_params, n_params + n_outs))
    in_specs = tuple(
        PartitionSpec(None) if name == "wrepl" else PartitionSpec("core")
        for name in in_names) + (PartitionSpec("core"),) * n_outs
    sharded = jax.jit(
        shard_map(_body, mesh=mesh, in_specs=in_specs,
                  out_specs=(PartitionSpec("core"),) * n_outs,
                  check_rep=False),
        donate_argnums=donate, keep_unused=True)

    xout_shape = out_shapes[0][0]

    def run(args):
        shards, wrepl = args
        ins = []
        for name in in_names:
            if name == "wrepl":
                ins.append(wrepl)
            else:
                ins.append(np.concatenate(shards, axis=0))
        zeros = np.zeros((NCORES * xout_shape[0], *xout_shape[1:]), np.float32)
        out_arrs = sharded(*ins, zeros)
        return np.asarray(out_arrs[0]).reshape(NCORES, *xout_shape)

    return run


# ---------------------------------------------------------------------------
# General fallback (arbitrary edge_index) — exact reference math via jax
# ---------------------------------------------------------------------------

def _fallback(inputs):
    import jax
    import jax.numpy as jnp

    def message_pass(x, edge_index, ew, wq, wk, wv, we, wo, bo):
        src, tgt = edge_index[0], edge_index[1]
        dout = wo.shape[1]
        heads = wq.shape[1] // dout
        q = (x[tgt] @ wq).reshape(-1, heads, dout)
        k = (x[src] @ wk).reshape(-1, heads, dout)
        v = (x[src] @ wv).reshape(-1, heads, dout)
        attn = (q * k).sum(-1) / jnp.sqrt(jnp.float32(dout)) + ew @ we
        attn = jax.nn.leaky_relu(attn, negative_slope=0.2)
        attn = jax.nn.softmax(attn, axis=0)
        weighted_v = (attn[:, :, None] * v).reshape(-1, heads * dout)
        out = jax.ops.segment_sum(weighted_v, tgt, num_segments=x.shape[0])
        return out @ wo + bo + x

    f = inputs
    x = jnp.asarray(f["node_features"], jnp.float32)
    x = x.at[:jnp.asarray(f["x_input"]).shape[0], 0].set(jnp.asarray(f["x_input"]))
    ei = jnp.asarray(f["edge_index"], jnp.int32)
    ew = jnp.asarray(f["edge_weights"], jnp.float32)
    x = jax.nn.relu(message_pass(x, ei, ew, f["wq1"], f["wk1"], f["wv1"],
                                 f["we1"], f["wo1"], f["bo1"]))
    x = jax.nn.relu(message_pass(x, ei, ew, f["wq2"], f["wk2"], f["wv2"],
                                 f["we2"], f["wo2"], f["bo2"]))
    n_out = int(f["num_output_nodes"])
    out_nodes = x[x.shape[0] - n_out:]
    output = jax.nn.sigmoid(out_nodes @ jnp.asarray(f["wproj"])
                            + jnp.asarray(f["bproj"])).squeeze()
    return np.asarray(output, np.float32), np.asarray(x, np.float32)


# ---------------------------------------------------------------------------

def _is_structured(inputs):
    try:
        ei = np.asarray(inputs["edge_index"])
        if ei.shape != (2, E):
            return False
        if int(inputs["num_output_nodes"]) != N_OUT:
            return False
        if np.asarray(inputs["node_features"]).shape != (N_NODES, D):
            return False
        return bool(np.array_equal(ei, _expected_edge_index()))
    except Exception:
        return False


def kernel(**inputs):
    if not _is_structured(inputs):
        return _fallback(inputs)
    run = _get_runner()
    results = run(_shard_inputs(inputs))
    return _unshard(results)


# revision 19
# speedup vs baseline: 3.4729x; 1.8632x over previous
"""Trainium2 Bass kernel for nn_DynamicGraphNet (2-layer GNN attention message passing).

Contract: kernel(**inputs) takes the FULL unsharded inputs (as produced by the
reference's setup_inputs) and returns the full output (output_vec[64], x[2176, 64]).

Strategy: the reference's edge_index is a deterministic complete-bipartite
block graph (64 input -> 2048 hidden -> 64 output).  We verify that structure
at runtime; when it holds, the gather/scatter collapses into dense per-head
matmuls.  The whole problem is tiny enough to live in one NeuronCore's SBUF,
and on-chip collectives cost ~200us fixed each, so the fastest layout is a
single-core kernel with no cross-core communication (measured ~2x faster
end-to-end than an 8-way shard paying two AllGathers per layer).  If the
structure check fails, a general jax fallback implements the exact reference
math for arbitrary graphs.

Layout notes:
- Node features are kept transposed (XT [64 feat, 2176 nodes]) with columns
  reordered to [hidden(2048) | input(64) | output(64)] so every 128-node tile
  is cleanly aligned.
- Block-1 logits (input->hidden edges) are packed [128, 4096]: partition
  p = src_i + 64*(head//2), free = (head%2)*2048 + tgt_j, so elementwise
  passes run at full 128-partition width.
- Block-2 logits (hidden->output edges) are packed [128, 4096]: partition
  p = tgt_j%128, free = head*1024 + (j//128)*64 + out_o.
- The global edge-softmax uses per-partition row maxima, folded back exactly
  via exp(rowmax - globalmax) scaling of the exp'd logits, with the global
  1/sum folded into the wo matrix's rows.  Math is exact (fp reassociation
  only).
- Hardware quirk found empirically: all matmuls accumulating into the same
  PSUM tile must use operands with the SAME base partition (mixing base 0/64
  operands within one PSUM tile's groups aborts the NEFF at runtime).
"""

import functools
import os
import sys

import numpy as np

for _p in ("/root/.axon_site", "/root/.axon_site/_ro/trn_rl_repo",
           "/root/.axon_site/_ro/pypackages", "/opt/trn_rl_repo"):
    if os.path.isdir(_p) and _p not in sys.path:
        sys.path.append(_p)

N_NODES = 2176
N_IN = 64
N_HID = 2048
N_OUT = 64
D = 64
H = 4
HD = D * H  # 256
E1 = N_IN * N_HID
E2 = N_HID * N_OUT
E = E1 + E2

# packed input blob layout (floats)
OFF_XT = 0                       # [64, 2176]
OFF_EW1 = D * N_NODES            # [64, 2048] block-1 edge weights (i-major)
OFF_EW2P = OFF_EW1 + N_IN * N_HID    # [128, 1024] block-2 packed
OFF_W = OFF_EW2P + 128 * 1024
LAYER_W = 4 * 16384 + H + D      # wq+wks+wv+wo, then we, bo
NBLOB = OFF_W + 2 * LAYER_W + D + 1


def _expected_edge_index() -> np.ndarray:
    hs, os_ = N_IN, N_IN + N_HID
    s1 = np.repeat(np.arange(0, N_IN), N_HID)
    t1 = np.tile(np.arange(hs, os_), N_IN)
    s2 = np.repeat(np.arange(hs, os_), N_OUT)
    t2 = np.tile(np.arange(os_, N_NODES), N_HID)
    return np.stack([np.concatenate([s1, s2]), np.concatenate([t1, t2])]).astype(np.int32)


# ---------------------------------------------------------------------------
# Bass program (single core)
# ---------------------------------------------------------------------------

def _build_program():
    import concourse.bacc as bacc
    import concourse.mybir as mybir
    from concourse import masks
    from concourse.tile import TileContext

    F32 = mybir.dt.float32
    AF = mybir.ActivationFunctionType
    ALU = mybir.AluOpType
    AX = mybir.AxisListType

    nc = bacc.Bacc("TRN2", target_bir_lowering=False, debug=False, num_devices=1)

    blob_d = nc.dram_tensor("blob", [1, NBLOB], F32, kind="ExternalInput")
    xout_d = nc.dram_tensor("xout", [D, N_NODES + 1], F32, kind="ExternalOutput")

    def bview(off, p, f):
        return blob_d[0:1, off:off + p * f].rearrange("a (p f) -> (a p) f", p=p)

    HIDC = slice(0, N_HID)
    IC = slice(N_HID, N_HID + N_IN)
    OC = slice(N_HID + N_IN, N_NODES)
    # N-chunks for 2176-wide matmul outputs (PSUM free <= 512)
    CH2176 = [(0, 512), (512, 512), (1024, 512), (1536, 512), (2048, 128)]
    CH2048 = [(0, 512), (512, 512), (1024, 512), (1536, 512)]

    with TileContext(nc) as tc:
        with (
            tc.tile_pool(name="consts", bufs=1) as cp,
            tc.tile_pool(name="wts", bufs=1) as wpool,
            tc.tile_pool(name="work", bufs=1) as wk,
            tc.tile_pool(name="ps", bufs=1, space="PSUM") as ps,
        ):
            # ---- constants ----
            ident = cp.tile([128, 128], F32)
            masks.make_identity(nc, ident[:])
            ones = cp.tile([1, 128], F32)
            nc.vector.memset(ones[:], 1.0)
            onesc = cp.tile([128, 1], F32)
            nc.vector.memset(onesc[:], 1.0)
            half = cp.tile([128, 2], F32)
            nc.vector.memset(half[:], 0.0)
            nc.vector.memset(half[0:64, 0:1], 1.0)
            nc.vector.memset(half[64:128, 1:2], 1.0)
            # head-expansion one-hots: expA[k, m]=1 iff m//64==k (k<2); expB k-2
            expA_cols = cp.tile([128, H], F32)
            expB_cols = cp.tile([128, H], F32)
            nc.vector.memset(expA_cols[:], 0.0)
            nc.vector.memset(expB_cols[:], 0.0)
            nc.vector.memset(expA_cols[0:64, 0:1], 1.0)
            nc.vector.memset(expA_cols[64:128, 1:2], 1.0)
            nc.vector.memset(expB_cols[0:64, 2:3], 1.0)
            nc.vector.memset(expB_cols[64:128, 3:4], 1.0)
            expA_p = ps.tile([H, 128], F32, tag="small", bufs=2, name="expA_p")
            nc.tensor.transpose(expA_p[:], expA_cols[:], ident[:])
            expA = cp.tile([H, 128], F32)
            nc.vector.tensor_copy(expA[:], expA_p[:])
            expB_p = ps.tile([H, 128], F32, tag="small", bufs=2, name="expB_p")
            nc.tensor.transpose(expB_p[:], expB_cols[:], ident[:])
            expB = cp.tile([H, 128], F32)
            nc.vector.tensor_copy(expB[:], expB_p[:])

            # ---- load inputs ----
            def load_from(view, shape, name):
                t = wpool.tile(shape, F32, tag=name, name=name)
                nc.sync.dma_start(out=t[:], in_=view)
                return t

            XT0 = load_from(bview(OFF_XT, D, N_NODES), [D, N_NODES], "xT")
            EW12 = wpool.tile([128, N_HID], F32, tag="ew12", name="ew12")
            nc.sync.dma_start(out=EW12[0:64, :], in_=bview(OFF_EW1, 64, N_HID))
            nc.sync.dma_start(out=EW12[64:128, :], in_=bview(OFF_EW1, 64, N_HID))
            EW2P = load_from(bview(OFF_EW2P, 128, 1024), [128, 1024], "ew2p")
            WQ, WKS, WV, WE, BO, WO0, WO1 = [], [], [], [], [], [], []
            for i in range(2):
                base = OFF_W + i * LAYER_W
                WQ.append(load_from(bview(base, D, HD), [D, HD], f"wq{i}"))
                WKS.append(load_from(bview(base + 16384, D, HD), [D, HD], f"wks{i}"))
                WV.append(load_from(bview(base + 32768, D, HD), [D, HD], f"wv{i}"))
                WO0.append(load_from(bview(base + 49152, 128, D), [128, D], f"wo{i}a"))
                WO1.append(load_from(bview(base + 49152 + 8192, 128, D), [128, D], f"wo{i}b"))
                WE.append(load_from(bview(base + 65536, 1, H), [1, H], f"we{i}"))
                BO.append(load_from(bview(base + 65540, D, 1), [D, 1], f"bo{i}"))
            WPRJ = load_from(bview(OFF_W + 2 * LAYER_W, D, 1), [D, 1], "wproj")
            BPRJ = load_from(bview(OFF_W + 2 * LAYER_W + D, 1, 1), [1, 1], "bproj")

            def layer(li, XT):
                wq, wks, wv, we, bo = WQ[li], WKS[li], WV[li], WE[li], BO[li]
                wo0, wo1 = WO0[li], WO1[li]

                # ---- per-head Q/K (transposed [64 feat, 2176 nodes]) ----
                QTh = [wk.tile([D, N_NODES], F32, tag=f"qth{h}", name=f"qth{h}")
                       for h in range(H)]
                KTh = [wk.tile([D, N_NODES], F32, tag=f"kth{h}", name=f"kth{h}")
                       for h in range(H)]
                for h in range(H):
                    for ci, (c0, cw) in enumerate(CH2176):
                        qp = ps.tile([D, 512], F32, tag="qkv", bufs=4,
                                     name=f"q_p{h}_{ci}")
                        nc.tensor.matmul(qp[:, 0:cw], wq[:, 64 * h:64 * h + 64],
                                         XT[:, c0:c0 + cw])
                        kp = ps.tile([D, 512], F32, tag="qkv", bufs=4,
                                     name=f"k_p{h}_{ci}")
                        nc.tensor.matmul(kp[:, 0:cw], wks[:, 64 * h:64 * h + 64],
                                         XT[:, c0:c0 + cw])
                        nc.vector.tensor_copy(QTh[h][:, c0:c0 + cw], qp[:, 0:cw])
                        nc.scalar.copy(KTh[h][:, c0:c0 + cw], kp[:, 0:cw])
                # V node-major [128, 17*256]: V[p, 256*t+f] = V(node 128t+p, f)
                V = wk.tile([128, 17 * HD], F32, tag="v", name="v")
                for t in range(17):
                    vp = ps.tile([128, HD], F32, tag="qkv", bufs=4, name=f"v_p{t}")
                    nc.tensor.matmul(vp[:], XT[:, 128 * t:128 * (t + 1)], wv[:])
                    if t % 2 == 0:
                        nc.vector.tensor_copy(V[:, HD * t:HD * (t + 1)], vp[:])
                    else:
                        nc.scalar.copy(V[:, HD * t:HD * (t + 1)], vp[:])

                # ---- W2[p, hh] = we[hh + 2*(p>=64)] ----
                web_p = ps.tile([128, H], F32, tag="small", bufs=2, name="web_p")
                nc.tensor.matmul(web_p[:], ones[:], we[:])
                web = wk.tile([128, H], F32, tag="web")
                nc.vector.tensor_copy(web[:], web_p[:])
                W2 = wk.tile([128, 2], F32, tag="w2")
                nc.vector.tensor_copy(W2[0:64, 0:1], web[0:64, 0:1])
                nc.vector.tensor_copy(W2[64:128, 0:1], web[64:128, 2:3])
                nc.vector.tensor_copy(W2[0:64, 1:2], web[0:64, 1:2])
                nc.vector.tensor_copy(W2[64:128, 1:2], web[64:128, 3:4])

                # ---- block-1 logits LA [128, 4096]: p=i+64*(h//2), f=(h%2)*2048+j
                LA = wk.tile([128, 2 * N_HID], F32, tag="la")
                for hh in range(2):
                    for c4, (c0, cw) in enumerate(CH2048):
                        sp = ps.tile([128, 512], F32, tag="sp", bufs=2,
                                     name=f"s1_{hh}_{c4}")
                        nc.tensor.matmul(sp[0:64, :],
                                         KTh[hh][:, IC], QTh[hh][:, c0:c0 + cw])
                        nc.tensor.matmul(sp[64:128, :],
                                         KTh[hh + 2][:, IC], QTh[hh + 2][:, c0:c0 + cw])
                        # LA = EW1*we + S  (fused)
                        nc.vector.scalar_tensor_tensor(
                            out=LA[:, 2048 * hh + c0:2048 * hh + c0 + cw],
                            in0=EW12[:, c0:c0 + cw], scalar=W2[:, hh:hh + 1],
                            in1=sp[:], op0=ALU.mult, op1=ALU.add)

                # ---- block-2 logits LB [128, 4096]: p=j%128, f=h*1024+(j//128)*64+o
                LB = wk.tile([128, 4096], F32, tag="lb")
                for h in range(H):
                    for cp8 in range(2):
                        sp2 = ps.tile([128, 512], F32, tag="sp", bufs=2,
                                      name=f"s2_{h}_{cp8}")
                        for c8 in range(8):
                            c = 8 * cp8 + c8
                            nc.tensor.matmul(
                                sp2[:, 64 * c8:64 * c8 + 64],
                                KTh[h][:, 128 * c:128 * (c + 1)], QTh[h][:, OC])
                        nc.vector.scalar_tensor_tensor(
                            out=LB[:, 1024 * h + 512 * cp8:1024 * h + 512 * (cp8 + 1)],
                            in0=EW2P[:, 512 * cp8:512 * (cp8 + 1)],
                            scalar=web[:, h:h + 1],
                            in1=sp2[:], op0=ALU.mult, op1=ALU.add)

                # ---- leaky relu (in place) ----
                nc.vector.scalar_tensor_tensor(out=LA[:], in0=LA[:], scalar=0.2,
                                               in1=LA[:], op0=ALU.mult, op1=ALU.max)
                nc.vector.scalar_tensor_tensor(out=LB[:], in0=LB[:], scalar=0.2,
                                               in1=LB[:], op0=ALU.mult, op1=ALU.max)

                # ---- per-partition (negated) row maxima ----
                nm1 = wk.tile([128, 2], F32, tag="nm1")
                nm2 = wk.tile([128, H], F32, tag="nm2")
                for hh in range(2):
                    nc.vector.tensor_reduce(
                        out=nm1[:, hh:hh + 1], in_=LA[:, 2048 * hh:2048 * (hh + 1)],
                        op=ALU.max, axis=AX.X, negate=True)
                for h in range(H):
                    nc.vector.tensor_reduce(
                        out=nm2[:, h:h + 1], in_=LB[:, 1024 * h:1024 * (h + 1)],
                        op=ALU.max, axis=AX.X, negate=True)
                m1 = wk.tile([128, 2], F32, tag="m1")
                m2 = wk.tile([128, H], F32, tag="m2")
                nc.vector.tensor_scalar_mul(m1[:], nm1[:], -1.0)
                nc.vector.tensor_scalar_mul(m2[:], nm2[:], -1.0)

                # ---- U = exp(L - rowmax), rowsums (in place) ----
                rs1 = wk.tile([128, 2], F32, tag="rs1")
                rs2 = wk.tile([128, H], F32, tag="rs2")
                for hh in range(2):
                    nc.scalar.activation(
                        LA[:, 2048 * hh:2048 * (hh + 1)],
                        LA[:, 2048 * hh:2048 * (hh + 1)], AF.Exp,
                        bias=nm1[:, hh:hh + 1], accum_out=rs1[:, hh:hh + 1])
                for h in range(H):
                    nc.scalar.activation(
                        LB[:, 1024 * h:1024 * (h + 1)],
                        LB[:, 1024 * h:1024 * (h + 1)], AF.Exp,
                        bias=nm2[:, h:h + 1], accum_out=rs2[:, h:h + 1])

                # ---- global softmax stats ----
                m1T_p = ps.tile([2, 128], F32, tag="small", bufs=2, name="m1T_p")
                nc.tensor.transpose(m1T_p[:], m1[:], ident[:])
                m2T_p = ps.tile([H, 128], F32, tag="small", bufs=2, name="m2T_p")
                nc.tensor.transpose(m2T_p[:], m2[:], ident[:])
                ra = wk.tile([2, 1], F32, tag="ra")
                rb = wk.tile([2, 1], F32, tag="rb")
                mb = wk.tile([H, 1], F32, tag="mb")
                nc.vector.tensor_reduce(out=ra[:], in_=m1T_p[:, 0:64],
                                        op=ALU.max, axis=AX.X)
                nc.vector.tensor_reduce(out=rb[:], in_=m1T_p[:, 64:128],
                                        op=ALU.max, axis=AX.X)
                nc.vector.tensor_reduce(out=mb[:], in_=m2T_p[:], op=ALU.max, axis=AX.X)
                raT_p = ps.tile([1, 2], F32, tag="small", bufs=2, name="raT_p")
                nc.tensor.transpose(raT_p[:], ra[:], ident[0:2, 0:2])
                rbT_p = ps.tile([1, 2], F32, tag="small", bufs=2, name="rbT_p")
                nc.tensor.transpose(rbT_p[:], rb[:], ident[0:2, 0:2])
                mbT_p = ps.tile([1, H], F32, tag="small", bufs=2, name="mbT_p")
                nc.tensor.transpose(mbT_p[:], mb[:], ident[0:4, 0:4])
                m1row = wk.tile([1, H], F32, tag="m1row")
                nc.vector.tensor_copy(m1row[:, 0:2], raT_p[:])
                nc.vector.tensor_copy(m1row[:, 2:4], rbT_p[:])
                mgrow = wk.tile([1, H], F32, tag="mgrow")
                nc.vector.tensor_max(mgrow[:], m1row[:], mbT_p[:])

                # broadcasts of the global max
                MG1_p = ps.tile([128, 2], F32, tag="small", bufs=2, name="MG1_p")
                nc.tensor.matmul(MG1_p[0:64, :], ones[:, 0:64], mgrow[:, 0:2])
                nc.tensor.matmul(MG1_p[64:128, :], ones[:, 0:64], mgrow[:, 2:4])
                MG2_p = ps.tile([128, H], F32, tag="small", bufs=2, name="MG2_p")
                nc.tensor.matmul(MG2_p[:], ones[:], mgrow[:])
                # E = exp(rowmax - globalmax)
                d1 = wk.tile([128, 2], F32, tag="d1")
                nc.vector.tensor_sub(d1[:], m1[:], MG1_p[:])
                E1 = wk.tile([128, 2], F32, tag="e1")
                nc.scalar.activation(E1[:], d1[:], AF.Exp)
                d2 = wk.tile([128, H], F32, tag="d2")
                nc.vector.tensor_sub(d2[:], m2[:], MG2_p[:])
                E2 = wk.tile([128, H], F32, tag="e2")
                nc.scalar.activation(E2[:], d2[:], AF.Exp)

                # global denominators S_g[h] (row vector [1, 4])
                w1 = wk.tile([128, 2], F32, tag="w1s")
                nc.vector.tensor_mul(w1[:], rs1[:], E1[:])
                w2s = wk.tile([128, H], F32, tag="w2s")
                nc.vector.tensor_mul(w2s[:], rs2[:], E2[:])
                s1a_p = ps.tile([1, 2], F32, tag="small", bufs=2, name="s1a_p")
                nc.tensor.matmul(s1a_p[:], half[:, 0:1], w1[:])
                s1b_p = ps.tile([1, 2], F32, tag="small", bufs=2, name="s1b_p")
                nc.tensor.matmul(s1b_p[:], half[:, 1:2], w1[:])
                s2r_p = ps.tile([1, H], F32, tag="small", bufs=2, name="s2r_p")
                nc.tensor.matmul(s2r_p[:], onesc[:], w2s[:])
                s1row = wk.tile([1, H], F32, tag="s1row")
                nc.vector.tensor_copy(s1row[:, 0:2], s1a_p[:])
                nc.vector.tensor_copy(s1row[:, 2:4], s1b_p[:])
                sgrow = wk.tile([1, H], F32, tag="sgrow")
                nc.vector.tensor_add(sgrow[:], s1row[:], s2r_p[:])
                sgcol_p = ps.tile([H, 1], F32, tag="small", bufs=2, name="sgcol_p")
                nc.tensor.transpose(sgcol_p[:], sgrow[:], ident[0:1, 0:1])
                sgcol = wk.tile([H, 1], F32, tag="sgcol")
                nc.vector.tensor_copy(sgcol[:], sgcol_p[:])
                sinv = wk.tile([H, 1], F32, tag="sinv")
                nc.vector.reciprocal(sinv[:], sgcol[:])
                # wo rows scaled by 1/S_g(head)
                rsc0_p = ps.tile([128, 1], F32, tag="small", bufs=2, name="rsc0_p")
                nc.tensor.matmul(rsc0_p[:], expA[:], sinv[:])
                rsc1_p = ps.tile([128, 1], F32, tag="small", bufs=2, name="rsc1_p")
                nc.tensor.matmul(rsc1_p[:], expB[:], sinv[:])
                rsc0 = wk.tile([128, 1], F32, tag="rsc0")
                rsc1 = wk.tile([128, 1], F32, tag="rsc1")
                nc.vector.tensor_copy(rsc0[:], rsc0_p[:])
                nc.vector.tensor_copy(rsc1[:], rsc1_p[:])
                wos0 = wk.tile([128, D], F32, tag="wos0")
                wos1 = wk.tile([128, D], F32, tag="wos1")
                nc.vector.tensor_scalar_mul(wos0[:], wo0[:], rsc0[:])
                nc.vector.tensor_scalar_mul(wos1[:], wo1[:], rsc1[:])

                # ---- fold exp(rowmax - globalmax) into U (in place) ----
                for hh in range(2):
                    nc.vector.tensor_scalar_mul(
                        LA[:, 2048 * hh:2048 * (hh + 1)],
                        LA[:, 2048 * hh:2048 * (hh + 1)], E1[:, hh:hh + 1])
                for h in range(H):
                    nc.vector.tensor_scalar_mul(
                        LB[:, 1024 * h:1024 * (h + 1)],
                        LB[:, 1024 * h:1024 * (h + 1)], E2[:, h:h + 1])

                # ---- aggregation ----
                # V_input duplicated on both partition halves
                VI2 = wk.tile([128, HD], F32, tag="vi2")
                nc.vector.tensor_copy(VI2[0:64, :], V[0:64, 16 * HD:17 * HD])
                nc.vector.tensor_copy(VI2[64:128, :], V[0:64, 16 * HD:17 * HD])
                # block1: A1S[pair][64*(h%2)+d, j] = sum_i Vhat_I[i, hd] * U1[i, hj]
                A1S = [wk.tile([128, N_HID], F32, tag=f"a1s{i}", name=f"a1s{i}")
                       for i in range(2)]
                for pair in range(2):
                    pb = 64 * pair  # ULA/VI2 partition base for this head pair
                    for c4, (c0, cw) in enumerate(CH2048):
                        ap = ps.tile([128, 512], F32, tag="sp", bufs=2,
                                     name=f"a1p{pair}_{c4}")
                        for hh in range(2):
                            h = hh + 2 * pair
                            nc.tensor.matmul(
                                ap[64 * hh:64 * hh + 64, 0:cw],
                                VI2[pb:pb + 64, 64 * h:64 * h + 64],
                                LA[pb:pb + 64, 2048 * hh + c0:2048 * hh + c0 + cw])
                        if c4 % 2 == 0:
                            nc.vector.tensor_copy(A1S[pair][:, c0:c0 + cw], ap[:, 0:cw])
                        else:
                            nc.scalar.copy(A1S[pair][:, c0:c0 + cw], ap[:, 0:cw])
                # block2: A2S[pair][64*(h%2)+d, o] = sum_j Vhat[j, hd] * U2[j, ho]
                A2S = [wk.tile([128, N_OUT], F32, tag=f"a2s{i}", name=f"a2s{i}")
                       for i in range(2)]
                for pair in range(2):
                    a2p = ps.tile([128, N_OUT], F32, tag="small", bufs=2,
                                  name=f"a2p{pair}")
                    for hh in range(2):
                        h = hh + 2 * pair
                        for c in range(16):
                            nc.tensor.matmul(
                                a2p[64 * hh:64 * hh + 64, :],
                                V[:, HD * c + 64 * h:HD * c + 64 * h + 64],
                                LB[:, 1024 * h + 64 * c:1024 * h + 64 * c + 64],
                                start=(c == 0), stop=(c == 15))
                    nc.vector.tensor_copy(A2S[pair][:], a2p[:])

                # ---- output projection + residual + relu ----
                X2 = wk.tile([D, N_NODES], F32, tag="x2", bufs=2)
                for c4, (c0, cw) in enumerate(CH2048):
                    xcp = ps.tile([D, 512], F32, tag="sp", bufs=2, name=f"xcp{c4}")
                    nc.tensor.matmul(xcp[:, 0:cw], wos0[:], A1S[0][:, c0:c0 + cw],
                                     start=True, stop=False)
                    nc.tensor.matmul(xcp[:, 0:cw], wos1[:], A1S[1][:, c0:c0 + cw],
                                     start=False, stop=True)
                    tC = wk.tile([D, 512], F32, tag="tc_res", bufs=2)
                    nc.vector.tensor_add(tC[:, 0:cw], xcp[:, 0:cw], XT[:, c0:c0 + cw])
                    nc.scalar.activation(X2[:, c0:c0 + cw], tC[:, 0:cw],
                                         AF.Relu, bias=bo[:])
                xop = ps.tile([D, N_OUT], F32, tag="small", bufs=2, name="xop")
                nc.tensor.matmul(xop[:], wos0[:], A2S[0][:], start=True, stop=False)
                nc.tensor.matmul(xop[:], wos1[:], A2S[1][:], start=False, stop=True)
                tO = wk.tile([D, N_OUT], F32, tag="to_res")
                nc.vector.tensor_add(tO[:], xop[:], XT[:, OC])
                nc.scalar.activation(X2[:, OC], tO[:], AF.Relu, bias=bo[:])
                nc.scalar.activation(X2[:, IC], XT[:, IC], AF.Relu, bias=bo[:])
                return X2

            def final(X3):
                prj_p = ps.tile([N_OUT, 1], F32, tag="small", bufs=2, name="prj_p")
                nc.tensor.matmul(prj_p[:], X3[:, OC], WPRJ[:])
                bpb_p = ps.tile([N_OUT, 1], F32, tag="small", bufs=2, name="bpb_p")
                nc.tensor.matmul(bpb_p[:], ones[:, 0:64], BPRJ[:])
                bpb = wk.tile([N_OUT, 1], F32, tag="bpb")
                nc.vector.tensor_copy(bpb[:], bpb_p[:])
                ovec = wk.tile([N_OUT, 1], F32, tag="ovec")
                nc.scalar.activation(ovec[:], prj_p[:],
                                     mybir.ActivationFunctionType.Sigmoid,
                                     bias=bpb[:])
                nc.sync.dma_start(out=xout_d[:, N_NODES:N_NODES + 1], in_=ovec[:])
                nc.sync.dma_start(out=xout_d[:, 0:N_NODES], in_=X3[:])

            loop_n = int(os.environ.get("GNN_LOOP", "1"))
            if loop_n > 1:
                with tc.For_i(0, loop_n, 1):
                    X2 = layer(0, XT0)
                    X3 = layer(1, X2)
                    final(X3)
            else:
                X2 = layer(0, XT0)
                X3 = layer(1, X2)
                final(X3)

    nc.compile()
    return nc


# ---------------------------------------------------------------------------
# Host-side packing / runner
# ---------------------------------------------------------------------------

def _pack_inputs(inputs):
    x = np.asarray(inputs["node_features"], np.float32).copy()
    x[:N_IN, 0] = np.asarray(inputs["x_input"], np.float32)
    xT = np.ascontiguousarray(x.T)  # [64, 2176] natural order
    # reorder cols to [hidden | input | output]
    xTr = np.concatenate([xT[:, N_IN:N_IN + N_HID], xT[:, :N_IN],
                          xT[:, N_IN + N_HID:]], axis=1)
    ew = np.asarray(inputs["edge_weights"], np.float32).reshape(-1)
    ew1 = ew[:E1].reshape(N_IN, N_HID)
    ew2 = ew[E1:].reshape(N_HID, N_OUT)
    # EW2P[p, c*64+o] = ew2[c*128+p, o]
    ew2p = ew2.reshape(16, 128, N_OUT).transpose(1, 0, 2).reshape(128, 1024)

    parts = [xTr.ravel(), ew1.ravel(), ew2p.ravel()]
    for l in (1, 2):
        parts += [
            np.asarray(inputs[f"wq{l}"], np.float32).ravel(),
            (np.asarray(inputs[f"wk{l}"], np.float32) * 0.125).ravel(),
            np.asarray(inputs[f"wv{l}"], np.float32).ravel(),
            np.asarray(inputs[f"wo{l}"], np.float32).ravel(),
            np.asarray(inputs[f"we{l}"], np.float32).ravel(),
            np.asarray(inputs[f"bo{l}"], np.float32).ravel(),
        ]
    parts += [np.asarray(inputs["wproj"], np.float32).ravel(),
              np.asarray(inputs["bproj"], np.float32).ravel()]
    blob = np.concatenate(parts)[None, :].astype(np.float32)
    assert blob.shape[1] == NBLOB, blob.shape
    return blob


def _unpack(xout):
    # xout [64, 2177]: cols 0:2048 hidden, 2048:2112 input, 2112:2176 output,
    # 2176 = sigmoid output vector
    x_full = np.empty((N_NODES, D), np.float32)
    x_full[N_IN:N_IN + N_HID] = xout[:, 0:N_HID].T
    x_full[:N_IN] = xout[:, N_HID:N_HID + N_IN].T
    x_full[N_IN + N_HID:] = xout[:, N_HID + N_IN:N_NODES].T
    out = xout[:, N_NODES].reshape(N_OUT).astype(np.float32)
    return out, x_full


@functools.lru_cache(maxsize=1)
def _get_runner():
    """Build + compile the program once; return callable(blob) -> xout."""
    import jax
    import concourse.mybir as mybir
    from concourse.bass2jax import (_bass_exec_p, install_neuronx_cc_hook,
                                    partition_id_tensor)

    nc = _build_program()
    install_neuronx_cc_hook()

    partition_name = nc.partition_id_tensor.name if nc.partition_id_tensor else None
    in_names, out_names, out_avals, out_shapes = [], [], [], []
    for alloc in nc.m.functions[0].allocations:
        if not isinstance(alloc, mybir.MemoryLocationSet):
            continue
        name = alloc.memorylocations[0].name
        if alloc.kind == "ExternalInput":
            if name != partition_name:
                in_names.append(name)
        elif alloc.kind == "ExternalOutput":
            out_names.append(name)
            shape = tuple(alloc.tensor_shape)
            dtype = mybir.dt.np(alloc.dtype)
            out_avals.append(jax.core.ShapedArray(shape, dtype))
            out_shapes.append((shape, dtype))
    assert in_names == ["blob"] and out_names == ["xout"], (in_names, out_names)
    in_names_all = list(in_names) + out_names + (
        [partition_name] if partition_name else [])

    def _body(*args):
        operands = list(args)
        if partition_name is not None:
            operands.append(partition_id_tensor())
        outs = _bass_exec_p.bind(
            *operands, out_avals=tuple(out_avals), in_names=tuple(in_names_all),
            out_names=tuple(out_names), lowering_input_output_aliases=(),
            sim_require_finite=False, sim_require_nnan=False, nc=nc)
        return tuple(outs)

    jitted = jax.jit(_body, donate_argnums=(1,), keep_unused=True)
    xout_shape = out_shapes[0][0]

    def run(blob):
        zeros = np.zeros(xout_shape, np.float32)
        out_arrs = jitted(blob, zeros)
        return np.asarray(out_arrs[0])

    return run


# ---------------------------------------------------------------------------
# General fallback (arbitrary edge_index) — exact reference math via jax
# ---------------------------------------------------------------------------

def _fallback(inputs):
    import jax
    import jax.numpy as jnp

    def message_pass(x, edge_index, ew, wq, wk, wv, we, wo, bo):
        src, tgt = edge_index[0], edge_index[1]
        dout = wo.shape[1]
        heads = wq.shape[1] // dout
        q = (x[tgt] @ wq).reshape(-1, heads, dout)
        k = (x[src] @ wk).reshape(-1, heads, dout)
        v = (x[src] @ wv).reshape(-1, heads, dout)
        attn = (q * k).sum(-1) / jnp.sqrt(jnp.float32(dout)) + ew @ we
        attn = jax.nn.leaky_relu(attn, negative_slope=0.2)
        attn = jax.nn.softmax(attn, axis=0)
        weighted_v = (attn[:, :, None] * v).reshape(-1, heads * dout)
        out = jax.ops.segment_sum(weighted_v, tgt, num_segments=x.shape[0])
        return out @ wo + bo + x

    f = inputs
    x = jnp.asarray(f["node_features"], jnp.float32)
    x = x.at[:jnp.asarray(f["x_input"]).shape[0], 0].set(jnp.asarray(f["x_input"]))
    ei = jnp.asarray(f["edge_index"], jnp.int32)
    ew = jnp.asarray(f["edge_weights"], jnp.float32)
    x = jax.nn.relu(message_pass(x, ei, ew, f["wq1"], f["wk1"], f["wv1"],
                                 f["we1"], f["wo1"], f["bo1"]))
    x = jax.nn.relu(message_pass(x, ei, ew, f["wq2"], f["wk2"], f["wv2"],
                                 f["we2"], f["wo2"], f["bo2"]))
    n_out = int(f["num_output_nodes"])
    out_nodes = x[x.shape[0] - n_out:]
    output = jax.nn.sigmoid(out_nodes @ jnp.asarray(f["wproj"])
                            + jnp.asarray(f["bproj"])).squeeze()
    return np.asarray(output, np.float32), np.asarray(x, np.float32)


# ---------------------------------------------------------------------------

def _is_structured(inputs):
    try:
        ei = np.asarray(inputs["edge_index"])
        if ei.shape != (2, E):
            return False
        if int(inputs["num_output_nodes"]) != N_OUT:
            return False
        if np.asarray(inputs["node_features"]).shape != (N_NODES, D):
            return False
        return bool(np.array_equal(ei, _expected_edge_index()))
    except Exception:
        return False


def kernel(**inputs):
    if not _is_structured(inputs):
        return _fallback(inputs)
    run = _get_runner()
    return _unpack(run(_pack_inputs(inputs)))
